# revision 5
# baseline (speedup 1.0000x reference)
"""TimeSformer block (temporal attn -> spatial attn -> MLP) on 8 trn2 cores, v2.

Data-parallel over B=8: each NeuronCore processes one batch element.
v2: all matmul operands bf16 (FWL weight loads), qk/v/oT images SBUF-resident,
temporal attention as K=64 head-slice matmuls + post-exp block-diag mask,
spatial attention per-frame, MLP blocked with SBUF h1. Residual stream fp32.
"""

import os
import sys
from contextlib import ExitStack

sys.path.insert(0, "/opt/trn_rl_repo")

import numpy as np
import ml_dtypes

import concourse.bass as bass
import concourse.mybir as mybir
import concourse.tile as tile
from concourse import bacc
from concourse.bass_utils import run_bass_kernel_spmd

F32 = mybir.dt.float32
BF16 = mybir.dt.bfloat16
FP8 = mybir.dt.float8e4
DR = mybir.MatmulPerfMode.DoubleRow
WS = 16.0      # fp8 weight pre-scale; absorbed via sel12 (attn out) or eviction
AF = mybir.ActivationFunctionType
ALU = mybir.AluOpType

C = 768
CO = 6           # C / 128
H = 12
D = 64
T = 8
G = 196          # h*w sequences
NT = G * T       # 1568 temporal tokens
NSEQ = 197       # spatial seq len (cls + 196)
NS = 8 * NSEQ    # 1576 spatial tokens
N = 1569
HID = 3072
HIDO = 24        # HID / 128
P = 128
NSP = 1584       # NS padded so fp8 image chunk stride is 16B-aligned
EPS = 1e-5
SCALE = D ** -0.5

PHASES = os.environ.get("KPHASES", "12345678")
KDEBUG = bool(int(os.environ.get("KDEBUG", "0")))


def _ceil(a, b):
    return (a + b - 1) // b


def ln_tile_bf16(nc, tmp, out_pool, x_t, rows, eps_t):
    """LayerNorm over free dim (768) of a [rows<=128, 768] f32 tile -> bf16."""
    stats = tmp.tile([P, 2, 6], F32, tag="ln_stats")
    for s in range(2):
        nc.vector.bn_stats(out=stats[:rows, s], in_=x_t[:rows, s * 384:(s + 1) * 384])
    mv = tmp.tile([P, 2], F32, tag="ln_mv")
    nc.vector.bn_aggr(out=mv[:rows], in_=stats[:rows])
    nc.scalar.activation(out=mv[:rows, 1:2], in_=mv[:rows, 1:2], func=AF.Sqrt,
                         bias=eps_t[:rows], scale=1.0)
    nc.vector.reciprocal(out=mv[:rows, 1:2], in_=mv[:rows, 1:2])
    xln = out_pool.tile([P, C], BF16, tag="ln_out")
    nc.gpsimd.tensor_scalar(out=xln[:rows], in0=x_t[:rows],
                            scalar1=mv[:rows, 0:1], scalar2=mv[:rows, 1:2],
                            op0=ALU.subtract, op1=ALU.mult)
    return xln


def qkv_common(nc, tc, ctx, src_rows_dma, ntok, w_qk, w_v, qk_img, v_img,
               v_tiles, ident, eps_t, xlnT):
    """LN -> transpose -> xlnT (c-major bf16); qk c-major image; v token-major
    per-head-padded image (ones in col 64 of each head's 65-wide slot).

    v_tiles: list of (row0, rows, tslot) token tiles for the v matmul/eviction.
    """
    wpool = ctx.enter_context(tc.tile_pool(name="w_qkv", bufs=1))
    lnp = ctx.enter_context(tc.tile_pool(name="lnp", bufs=4))
    lnout = ctx.enter_context(tc.tile_pool(name="lnout", bufs=4))
    tpp = ctx.enter_context(tc.tile_pool(name="tp_psum", bufs=2, space="PSUM"))
    mmp = ctx.enter_context(tc.tile_pool(name="qkv_psum", bufs=4, space="PSUM"))

    wqk = wpool.tile([P, CO, 2 * C], FP8)
    nc.gpsimd.dma_start(wqk, w_qk[:].rearrange("(co p) o -> p co o", p=P))
    wv = wpool.tile([P, CO, C], FP8)
    nc.gpsimd.dma_start(wv, w_v[:].rearrange("(co p) o -> p co o", p=P))

    ntiles = _ceil(ntok, P)
    for i in range(ntiles):
        rows = min(P, ntok - i * P)
        x_t = lnp.tile([P, C], F32, tag="x_t")
        src_rows_dma(x_t, i, rows)
        xln = ln_tile_bf16(nc, lnp, lnout, x_t, rows, eps_t)
        ptg = tpp.tile([P, CO, P], BF16, tag="tp_psum")
        for co in range(CO):
            nc.tensor.matmul(ptg[:, co, :rows], xln[:rows, co * P:(co + 1) * P],
                             ident[:rows, :rows], is_transpose=True,
                             start=(co == 0), stop=(co == CO - 1))
        nc.vector.tensor_copy(out=xlnT[:, :, i * P:i * P + rows],
                              in_=ptg[:, :, :rows])

    # q,k c-major: image chunks 0..5 = q, 6..11 = k  (fp8 DoubleRow, K=256/chain)
    for b in range(_ceil(ntok, 512)):
        cols = min(512, ntok - b * 512)
        for o in range(12):
            ps = mmp.tile([P, 512], F32, tag="mm_ps")
            for ch in range(3):
                nc.tensor.matmul(ps[:, :cols],
                                 wqk[:, 2 * ch:2 * ch + 2, o * P:(o + 1) * P],
                                 xlnT[:, 2 * ch:2 * ch + 2, b * 512:b * 512 + cols],
                                 start=(ch == 0), stop=(ch == 2), perf_mode=DR)
            nc.scalar.activation(out=qk_img[:, o, b * 512:b * 512 + cols],
                                 in_=ps[:, :cols], func=AF.Copy, scale=1.0 / WS)
    # v token-major per-head-padded
    for tslot, rows, stat_fn in v_tiles:
        for half, fcols in ((0, 512), (1, 256)):
            ps = mmp.tile([P, 512], F32, tag="mm_ps")
            for ch in range(3):
                # stationary = xlnT chunk [c,2,tok], moving = wv [c,2,out]
                nc.tensor.matmul(
                    ps[:rows, :fcols], stat_fn(ch),
                    wv[:, 2 * ch:2 * ch + 2, half * 512:half * 512 + fcols],
                    start=(ch == 0), stop=(ch == 2), perf_mode=DR)
            # scatter head chunks into 65-strided per-head slots
            h0 = half * 8
            nc.scalar.activation(
                out=v_img[:rows, tslot, h0:h0 + fcols // D, 0:D],
                in_=ps[:rows, :fcols].rearrange("p (h d) -> p h d", d=D),
                func=AF.Copy, scale=1.0 / WS)


def temporal_attn(nc, tc, qk_img, v_img, mask_bd, sel_sb, oT_img):
    with ExitStack() as ctx:
        sp = ctx.enter_context(tc.tile_pool(name="t_spsum", bufs=3, space="PSUM"))
        op = ctx.enter_context(tc.tile_pool(name="t_opsum", bufs=2, space="PSUM"))
        pp = ctx.enter_context(tc.tile_pool(name="t_p", bufs=6))
        sgp = ctx.enter_context(tc.tile_pool(name="t_sg", bufs=2))
        sig = ctx.enter_context(tc.tile_pool(name="t_sig", bufs=1))
        bcp = ctx.enter_context(tc.tile_pool(name="t_bc", bufs=2, space="PSUM"))

        sigma = sig.tile([12, NT], F32)
        rinv = sig.tile([12, NT], BF16)
        nst = _ceil(NT, P)  # 13 subtiles, grouped 4 per PSUM bank
        groups = []
        st = 0
        while st < nst:
            take = min(4, nst - st)
            g = [(s, min(P, NT - s * P)) for s in range(st, st + take)]
            groups.append((st * P, g, (take - 1) * P + g[-1][1]))
            st += take
        for h in range(H):
            hp = (h % 2) * D
            ch = h // 2
            sg_h = sgp.tile([1, NT], F32, tag="sg_h")
            for col0, g, cols in groups:
                ps = sp.tile([P, 512], F32, tag="s_ps")
                for j, (st_, rows) in enumerate(g):
                    sl0 = st_ * P
                    nc.tensor.matmul(ps[:rows, j * P:j * P + rows],
                                     qk_img[hp:hp + D, 6 + ch, sl0:sl0 + rows],
                                     qk_img[hp:hp + D, ch, sl0:sl0 + rows],
                                     start=(j == 0), stop=(j == len(g) - 1))
                rmax = g[0][1]
                p_t = pp.tile([P, 512], BF16, tag="p_t")
                nc.scalar.activation(out=p_t[:rmax, :cols], in_=ps[:rmax, :cols],
                                     func=AF.Exp, scale=SCALE)
                nc.gpsimd.tensor_mul(out=p_t[:rmax, :cols], in0=p_t[:rmax, :cols],
                                      in1=mask_bd[:rmax, :cols])
                po = op.tile([D + 1, 512], F32, tag="o_ps")
                for j, (st_, rows) in enumerate(g):
                    nc.tensor.matmul(po[:, j * P:j * P + rows],
                                     v_img[:rows, st_, h, :],
                                     p_t[:rows, j * P:j * P + rows],
                                     start=(j == 0), stop=(j == len(g) - 1))
                nc.vector.tensor_copy(out=oT_img[hp:hp + D, ch, col0:col0 + cols],
                                      in_=po[0:D, :cols])
                nc.scalar.activation(out=sg_h[:, col0:col0 + cols],
                                     in_=po[D:D + 1, :cols], func=AF.Copy,
                                     scale=1.0)
            nc.sync.dma_start(sigma[h:h + 1], sg_h)
        with nc.allow_low_precision(reason="rinv feeds bf16 bcast matmul"):
            nc.vector.reciprocal(out=rinv, in_=sigma)
        for pr in range(6):
            for b in range(_ceil(NT, 512)):
                cols = min(512, NT - b * 512)
                bc = bcp.tile([P, 512], F32, tag="bc_ps")
                nc.tensor.matmul(bc[:, :cols], sel_sb[:, pr * P:(pr + 1) * P],
                                 rinv[:, b * 512:b * 512 + cols],
                                 start=True, stop=True)
                nc.vector.tensor_mul(
                    out=oT_img[:, pr, b * 512:b * 512 + cols],
                    in0=oT_img[:, pr, b * 512:b * 512 + cols],
                    in1=bc[:, :cols])


def temporal_proj(nc, tc, x_in, w_proj_t, w_tfc, oT_img, projT, xs_d):
    with ExitStack() as ctx:
        wp = ctx.enter_context(tc.tile_pool(name="p3_w", bufs=1))
        mp = ctx.enter_context(tc.tile_pool(name="p3_ps", bufs=4, space="PSUM"))
        tp = ctx.enter_context(tc.tile_pool(name="p3_t", bufs=3))

        wproj = wp.tile([P, CO, C], FP8)
        nc.sync.dma_start(wproj, w_proj_t[:].rearrange("(co p) o -> p co o", p=P))
        wtfc = wp.tile([P, CO, C], FP8)
        nc.sync.dma_start(wtfc, w_tfc[:].rearrange("(co p) o -> p co o", p=P))

        # xs_g rows 0..7 = x[0] (cls) for every frame; rows 8.. = xt (g-major)
        cls_sb = tp.tile([8, C], F32, tag="cls_sb")
        nc.gpsimd.dma_start(cls_sb, bass.AP(tensor=x_in, offset=0,
                                            ap=[[0, 8], [1, C]]))
        nc.sync.dma_start(xs_d[0:8, :], cls_sb)

        # projT = w_proj.T @ oT (c-major)
        for b in range(_ceil(NT, 512)):
            cols = min(512, NT - b * 512)
            for o in range(CO):
                ps = mp.tile([P, 512], F32, tag="p3ps")
                for ch in range(3):
                    nc.tensor.matmul(ps[:, :cols],
                                     wproj[:, 2 * ch:2 * ch + 2, o * P:(o + 1) * P],
                                     oT_img[:, 2 * ch:2 * ch + 2, b * 512:b * 512 + cols],
                                     start=(ch == 0), stop=(ch == 2), perf_mode=DR)
                nc.scalar.activation(out=projT[:, o, b * 512:b * 512 + cols],
                                     in_=ps[:, :cols], func=AF.Copy, scale=0.25)
        # xt = x + projT.T @ w_tfc, scattered to xs_d (t g c)
        for it in range(_ceil(NT, P)):
            tok0 = it * P
            rows = min(P, NT - tok0)
            x_t = tp.tile([P, C], F32, tag="x_t3")
            nc.sync.dma_start(x_t[:rows], x_in[1 + tok0:1 + tok0 + rows, :])
            xt = tp.tile([P, C], F32, tag="xt3")
            for half, fcols in ((0, 512), (1, 256)):
                ps = mp.tile([P, 512], F32, tag="p3ps")
                for ch in range(3):
                    nc.tensor.matmul(
                        ps[:rows, :fcols],
                        projT[:, 2 * ch:2 * ch + 2, tok0:tok0 + rows],
                        wtfc[:, 2 * ch:2 * ch + 2, half * 512:half * 512 + fcols],
                        start=(ch == 0), stop=(ch == 2), perf_mode=DR)
                nc.vector.tensor_add(
                    out=xt[:rows, half * 512:half * 512 + fcols],
                    in0=ps[:rows, :fcols],
                    in1=x_t[:rows, half * 512:half * 512 + fcols])
            nc.gpsimd.dma_start(xs_d[8 + tok0:8 + tok0 + rows, :], xt[:rows])


def spatial_attn(nc, tc, qk_img, v_img, sel_sb, oT_img):
    """g-major spatial attention: frame f = columns f::8 of the (j f) layout."""
    with ExitStack() as ctx:
        sp = ctx.enter_context(tc.tile_pool(name="s_spsum", bufs=2, space="PSUM"))
        op = ctx.enter_context(tc.tile_pool(name="s_opsum", bufs=2, space="PSUM"))
        pp = ctx.enter_context(tc.tile_pool(name="s_p", bufs=6))
        sgp = ctx.enter_context(tc.tile_pool(name="s_sg", bufs=2))
        sig = ctx.enter_context(tc.tile_pool(name="s_sig", bufs=1))
        bcp = ctx.enter_context(tc.tile_pool(name="s_bc", bufs=2, space="PSUM"))

        qk_r = qk_img[:].rearrange("p o (j f) -> p o j f", f=8)
        oT_r = oT_img[:].rearrange("p o (j f) -> p o j f", f=8)  # j=198 padded
        sigma = sig.tile([12, NS], F32)
        rinv = sig.tile([12, NS], BF16)
        CH1 = NSEQ - P  # 69
        for h in range(H):
            hp = (h % 2) * D
            ch = h // 2
            sg_h = sgp.tile([1, NS], F32, tag="sg_hs")
            sg_r = sg_h[:].rearrange("p (j f) -> p j f", f=8)
            for fp in range(4):
                f0 = 2 * fp
                ps0 = sp.tile([P, 2, NSEQ], F32, tag="s_ps0")
                ps1 = sp.tile([P, 2, NSEQ], F32, tag="s_ps1")
                for fi in range(2):
                    f = f0 + fi
                    q_sl = qk_r[hp:hp + D, ch, :, f]
                    nc.tensor.matmul(
                        ps0[:, fi], qk_r[hp:hp + D, 6 + ch, 0:P, f], q_sl,
                        start=(fi == 0), stop=(fi == 1))
                    nc.tensor.matmul(
                        ps1[:CH1, fi], qk_r[hp:hp + D, 6 + ch, P:NSEQ, f],
                        q_sl, start=(fi == 0), stop=(fi == 1))
                p0 = pp.tile([P, 2, NSEQ], BF16, tag="p_s0")
                p1 = pp.tile([P, 2, NSEQ], BF16, tag="p_s1")
                nc.scalar.activation(out=p0, in_=ps0, func=AF.Exp, scale=SCALE)
                nc.scalar.activation(out=p1[:CH1], in_=ps1[:CH1],
                                     func=AF.Exp, scale=SCALE)
                po = op.tile([D + 1, 2, NSEQ], F32, tag="o_ps_s")
                mms = [(fi, ci) for fi in range(2) for ci in range(2)]
                for idx, (fi, ci) in enumerate(mms):
                    f = f0 + fi
                    chlen = P if ci == 0 else CH1
                    psrc = p0 if ci == 0 else p1
                    nc.tensor.matmul(po[:, fi], v_img[:chlen, 2 * f + ci, h, :],
                                     psrc[:chlen, fi],
                                     start=(idx == 0), stop=(idx == len(mms) - 1))
                nc.vector.tensor_copy(
                    out=oT_r[hp:hp + D, ch, 0:NSEQ, f0:f0 + 2],
                    in_=po[0:D].rearrange("p f j -> p j f"))
                nc.scalar.activation(out=sg_r[:, :, f0:f0 + 2],
                                     in_=po[D:D + 1].rearrange("p f j -> p j f"),
                                     func=AF.Copy, scale=1.0)
            nc.sync.dma_start(sigma[h:h + 1], sg_h)
        with nc.allow_low_precision(reason="rinv feeds bf16 bcast matmul"):
            nc.vector.reciprocal(out=rinv, in_=sigma)
        for pr in range(6):
            for b in range(_ceil(NS, 512)):
                cols = min(512, NS - b * 512)
                bc = bcp.tile([P, 512], F32, tag="bc_s")
                nc.tensor.matmul(bc[:, :cols], sel_sb[:, pr * P:(pr + 1) * P],
                                 rinv[:, b * 512:b * 512 + cols],
                                 start=True, stop=True)
                nc.vector.tensor_mul(
                    out=oT_img[:, pr, b * 512:b * 512 + cols],
                    in0=oT_img[:, pr, b * 512:b * 512 + cols],
                    in1=bc[:, :cols])


def spatial_proj(nc, tc, xs_d, w_proj_s, oT_img, y_s):
    with ExitStack() as ctx:
        wp = ctx.enter_context(tc.tile_pool(name="p6_w", bufs=1))
        mp = ctx.enter_context(tc.tile_pool(name="p6_ps", bufs=4, space="PSUM"))
        tp = ctx.enter_context(tc.tile_pool(name="p6_t", bufs=3))
        wproj = wp.tile([P, CO, C], FP8)
        nc.sync.dma_start(wproj, w_proj_s[:].rearrange("(co p) o -> p co o", p=P))
        for i in range(_ceil(NS, P)):
            rows = min(P, NS - i * P)
            x_t = tp.tile([P, C], F32, tag="x_t6")
            nc.gpsimd.dma_start(x_t[:rows], xs_d[i * P:i * P + rows, :])
            yt = tp.tile([P, C], F32, tag="yt6")
            for half, fcols in ((0, 512), (1, 256)):
                ps = mp.tile([P, 512], F32, tag="p6ps")
                for ch in range(3):
                    nc.tensor.matmul(
                        ps[:rows, :fcols],
                        oT_img[:, 2 * ch:2 * ch + 2, i * P:i * P + rows],
                        wproj[:, 2 * ch:2 * ch + 2, half * 512:half * 512 + fcols],
                        start=(ch == 0), stop=(ch == 2), perf_mode=DR)
                nc.vector.tensor_add(
                    out=yt[:rows, half * 512:half * 512 + fcols],
                    in0=ps[:rows, :fcols],
                    in1=x_t[:rows, half * 512:half * 512 + fcols])
            nc.gpsimd.dma_start(y_s[i * P:i * P + rows, :], yt[:rows])


def cls_mean(nc, tc, y_s, one8, cls_row):
    with ExitStack() as ctx:
        tp = ctx.enter_context(tc.tile_pool(name="p7_t", bufs=1))
        mp = ctx.enter_context(tc.tile_pool(name="p7_ps", bufs=2, space="PSUM"))
        ycls = tp.tile([8, C], F32)
        nc.sync.dma_start(ycls, y_s[0:8, :])
        o8 = tp.tile([8, 1], F32)
        nc.sync.dma_start(o8, one8[:])
        res = tp.tile([1, C], F32)
        for half, fcols in ((0, 512), (1, 256)):
            ps = mp.tile([1, 512], F32, tag="p7_ps")
            nc.tensor.matmul(ps[:, :fcols], o8,
                             ycls[:, half * 512:half * 512 + fcols],
                             start=True, stop=True)
            nc.vector.tensor_copy(out=res[:, half * 512:half * 512 + fcols],
                                  in_=ps[:, :fcols])
        nc.sync.dma_start(cls_row[:], res)


def mlp(nc, tc, y_s, cls_row, wfc1, wfc2, out, ident, eps_t):
    blocks = [(0, 512), (512, 512), (1024, 512), (1536, 33)]

    def load_x(pool, tok0, it, btok):
        rows = min(P, btok - it * P)
        x_t = pool.tile([P, C], F32, tag="x_t8")
        r0 = 8 + tok0 + it * P
        if btok == 33:
            nc.gpsimd.dma_start(x_t[:32], y_s[r0:r0 + 32, :])
            nc.gpsimd.dma_start(x_t[32:33], cls_row[:])
        else:
            nc.gpsimd.dma_start(x_t[:rows], y_s[r0:r0 + rows, :])
        return x_t, rows

    with ExitStack() as ctx:
        lnp = ctx.enter_context(tc.tile_pool(name="p8_ln", bufs=3))
        lnout = ctx.enter_context(tc.tile_pool(name="p8_lno", bufs=3))
        tpp = ctx.enter_context(tc.tile_pool(name="p8_tp", bufs=2, space="PSUM"))
        xlp = ctx.enter_context(tc.tile_pool(name="p8_xlT", bufs=2))
        m1p = ctx.enter_context(tc.tile_pool(name="p8_ps1", bufs=2, space="PSUM"))
        h1p = ctx.enter_context(tc.tile_pool(name="p8_h1", bufs=2))
        m2p = ctx.enter_context(tc.tile_pool(name="p8_ps2", bufs=2, space="PSUM"))
        xrp = ctx.enter_context(tc.tile_pool(name="p8_xr", bufs=5))
        otp = ctx.enter_context(tc.tile_pool(name="p8_o", bufs=3))

        for tok0, btok in blocks:
            btok_mm = btok + (btok % 2)
            xlnT = xlp.tile([P, CO, 512], BF16, tag="xlnT8")
            if btok % 2:
                nc.vector.memset(xlnT.bitcast(F32), 0.0)
            xts = []
            for it in range(_ceil(btok, P)):
                x_t, rows = load_x(xrp, tok0, it, btok)
                xts.append((x_t, rows))
                xln = ln_tile_bf16(nc, lnp, lnout, x_t, rows, eps_t)
                ptg = tpp.tile([P, CO, P], BF16, tag="tp8")
                for co in range(CO):
                    nc.tensor.matmul(ptg[:, co, :rows],
                                     xln[:rows, co * P:(co + 1) * P],
                                     ident[:rows, :rows], is_transpose=True,
                                     start=(co == 0), stop=(co == CO - 1))
                nc.vector.tensor_copy(out=xlnT[:, :, it * P:it * P + rows],
                                      in_=ptg[:, :, :rows])
            # fc1 + gelu -> h1 c-major bf16
            h1t = h1p.tile([P, HIDO, 512], BF16, tag="h1t")
            for o in range(HIDO):
                ps = m1p.tile([P, 512], F32, tag="p8ps1")
                for co in range(CO):
                    nc.tensor.matmul(ps[:, :btok_mm], wfc1[:, co, o * P:(o + 1) * P],
                                     xlnT[:, co, :btok_mm],
                                     start=(co == 0), stop=(co == CO - 1))
                nc.scalar.activation(out=h1t[:, o, :btok], in_=ps[:, :btok],
                                     func=AF.Gelu)
            # fc2 token-major + residual
            for it in range(_ceil(btok, P)):
                x_t, rows = xts[it]
                o_t = otp.tile([P, C], F32, tag="o_t8")
                for half, fcols in ((0, 512), (1, 256)):
                    ps = m2p.tile([P, 512], F32, tag="p8ps2")
                    for ho in range(HIDO):
                        nc.tensor.matmul(
                            ps[:rows, :fcols], h1t[:, ho, it * P:it * P + rows],
                            wfc2[:, ho, half * 512:half * 512 + fcols],
                            start=(ho == 0), stop=(ho == HIDO - 1))
                    nc.vector.tensor_add(
                        out=o_t[:rows, half * 512:half * 512 + fcols],
                        in0=ps[:rows, :fcols],
                        in1=x_t[:rows, half * 512:half * 512 + fcols])
                row0 = tok0 + it * P
                if btok == 33:
                    nc.gpsimd.dma_start(out[1 + row0:1 + row0 + 32, :], o_t[:32])
                    nc.gpsimd.dma_start(out[0:1, :], o_t[32:33])
                else:
                    nc.gpsimd.dma_start(out[1 + row0:1 + row0 + rows, :],
                                        o_t[:rows])


def build_nc():
    nc = bacc.Bacc("TRN2", target_bir_lowering=False, debug=False)

    x_in = nc.dram_tensor("x", (N, C), F32, kind="ExternalInput")
    w_qk_t = nc.dram_tensor("w_qk_t", (C, 2 * C), FP8, kind="ExternalInput")
    w_v_t = nc.dram_tensor("w_v_t", (C, C), FP8, kind="ExternalInput")
    w_qk_s = nc.dram_tensor("w_qk_s", (C, 2 * C), FP8, kind="ExternalInput")
    w_v_s = nc.dram_tensor("w_v_s", (C, C), FP8, kind="ExternalInput")
    w_proj_t = nc.dram_tensor("w_proj_t", (C, C), FP8, kind="ExternalInput")
    w_tfc = nc.dram_tensor("w_tfc", (C, C), FP8, kind="ExternalInput")
    w_proj_s = nc.dram_tensor("w_proj_s", (C, C), FP8, kind="ExternalInput")
    w_fc1 = nc.dram_tensor("w_fc1", (C, HID), BF16, kind="ExternalInput")
    w_fc2 = nc.dram_tensor("w_fc2", (HID, C), BF16, kind="ExternalInput")
    mask_in = nc.dram_tensor("mask_bd", (P, 512), BF16, kind="ExternalInput")
    ident_in = nc.dram_tensor("ident", (P, P), BF16, kind="ExternalInput")
    sel12 = nc.dram_tensor("sel12", (12, C), BF16, kind="ExternalInput")
    one8 = nc.dram_tensor("one8", (8, 1), F32, kind="ExternalInput")
    out = nc.dram_tensor("out", (N, C), F32, kind="ExternalOutput")
    dbg = {}
    if KDEBUG:
        for nm, shp, dt_ in (("d_qk_t", (P, 12, NT), BF16),
                             ("d_v_t", (P, 16, 12, D + 1), BF16),
                             ("d_oT_t", (P, CO, NT), FP8),
                             ("d_xs", (NS, C), F32),
                             ("d_qk_s", (P, 12, NS), BF16),
                             ("d_oT_s", (P, CO, NS), FP8),
                             ("d_y_s", (NS, C), F32),
                             ("d_cls", (1, C), F32)):
            dbg[nm] = nc.dram_tensor(nm, shp, dt_, kind="ExternalOutput")

    # v tile specs are built inside build (need xlnT handle for stationary fns)

    with tile.TileContext(nc) as tc:
        with tc.tile_pool(name="dram", bufs=1, space="DRAM") as dram, \
             tc.tile_pool(name="const", bufs=1) as const:
            xs_d = dram.tile([NS, C], F32)
            y_s = dram.tile([NS, C], F32)
            cls_row = dram.tile([1, C], F32)

            ident = const.tile([P, P], BF16)
            nc.sync.dma_start(ident, ident_in[:])
            mask_bd = const.tile([P, 512], BF16)
            nc.sync.dma_start(mask_bd, mask_in[:])
            eps_t = const.tile([P, 1], F32)
            nc.vector.memset(eps_t, EPS)
            sel_sb = const.tile([12, C], BF16)
            nc.sync.dma_start(sel_sb, sel12[:])

            wfc1 = const.tile([P, CO, HID], BF16)
            wfc2 = const.tile([P, HIDO, C], BF16)
            nc.gpsimd.dma_start(wfc1, w_fc1[:].rearrange("(co p) o -> p co o", p=P))
            nc.gpsimd.dma_start(wfc2, w_fc2[:].rearrange("(ho p) o -> p ho o", p=P))

            img_ctx = ExitStack()
            img = img_ctx.enter_context(tc.tile_pool(name="img", bufs=1))
            qk_img = img.tile([P, 12, NS], BF16)
            v_img = img.tile([P, 16, 12, D + 1], BF16)
            oT_img = img.tile([P, CO, NSP], FP8)
            xlnT = img.tile([P, CO, NSP], FP8)
            projT = img.tile([P, CO, NT], FP8)

            # ones column in every v slot (col D of each head slot)
            if KDEBUG:
                nc.vector.memset(v_img, 1.0)
            else:
                nc.vector.memset(v_img[:, :, :, D:D + 1], 1.0)

            def mk_stat(row0, rows):
                return lambda ch: xlnT[:, 2 * ch:2 * ch + 2, row0:row0 + rows]

            xlnT_r = xlnT[:].rearrange("p o (j f) -> p o j f", f=8)  # j=198 padded

            def mk_stat_f(f, c0, chlen):
                return lambda ch: xlnT_r[:, 2 * ch:2 * ch + 2, c0:c0 + chlen, f]

            v_tiles_t = [(i, min(P, NT - i * P), mk_stat(i * P, min(P, NT - i * P)))
                         for i in range(_ceil(NT, P))]
            v_tiles_s = []
            for f in range(8):
                v_tiles_s.append((2 * f, P, mk_stat_f(f, 0, P)))
                v_tiles_s.append((2 * f + 1, NSEQ - P, mk_stat_f(f, P, NSEQ - P)))

            if "1" in PHASES:
                with ExitStack() as ctx:
                    def src_t(x_t, i, rows):
                        nc.sync.dma_start(x_t[:rows],
                                          x_in[1 + i * P:1 + i * P + rows, :])
                    qkv_common(nc, tc, ctx, src_t, NT, w_qk_t, w_v_t,
                               qk_img, v_img, v_tiles_t, ident, eps_t, xlnT)
            if "2" in PHASES:
                temporal_attn(nc, tc, qk_img, v_img, mask_bd, sel_sb, oT_img)
                if KDEBUG:
                    nc.sync.dma_start(dbg["d_oT_t"][:], oT_img[:, :, :NT])
            if KDEBUG and "1" in PHASES:
                nc.sync.dma_start(dbg["d_qk_t"][:], qk_img[:, :, :NT])
                nc.sync.dma_start(dbg["d_v_t"][:], v_img)
            if "3" in PHASES:
                temporal_proj(nc, tc, x_in, w_proj_t, w_tfc, oT_img, projT, xs_d)
                if KDEBUG:
                    nc.sync.dma_start(dbg["d_xs"][:], xs_d[:])
            if "4" in PHASES:
                with ExitStack() as ctx:
                    def src_s(x_t, i, rows):
                        nc.sync.dma_start(x_t[:rows],
                                          xs_d[i * P:i * P + rows, :])
                    qkv_common(nc, tc, ctx, src_s, NS, w_qk_s, w_v_s,
                               qk_img, v_img, v_tiles_s, ident, eps_t, xlnT)
                if KDEBUG:
                    nc.sync.dma_start(dbg["d_qk_s"][:], qk_img)
            if "5" in PHASES:
                spatial_attn(nc, tc, qk_img, v_img, sel_sb, oT_img)
                if KDEBUG:
                    nc.sync.dma_start(dbg["d_oT_s"][:], oT_img[:, :, :NS])
            if "6" in PHASES:
                spatial_proj(nc, tc, xs_d, w_proj_s, oT_img, y_s)
                if KDEBUG:
                    nc.sync.dma_start(dbg["d_y_s"][:], y_s[:])
            img_ctx.close()
            if "7" in PHASES:
                cls_mean(nc, tc, y_s, one8, cls_row)
                if KDEBUG:
                    nc.sync.dma_start(dbg["d_cls"][:], cls_row[:])
            if "8" in PHASES:
                mlp(nc, tc, y_s, cls_row, wfc1, wfc2, out, ident, eps_t)

    nc.compile()
    return nc


_NC_CACHE = None


def _get_nc():
    global _NC_CACHE
    if _NC_CACHE is None:
        _NC_CACHE = build_nc()
    return _NC_CACHE


def make_consts():
    idx = np.arange(P)
    mask = (idx[:, None] // T == idx[None, :] // T).astype(np.float32)
    mask = np.tile(mask, (1, 4))
    ident = np.eye(P, dtype=np.float32)
    sel = np.zeros((12, C), np.float32)
    for pr in range(6):
        for p in range(P):
            sel[2 * pr + p // D, pr * P + p] = 1.0 / WS
    one8 = np.full((8, 1), 0.125, np.float32)
    return mask, ident, sel, one8


def host_inputs(inputs):
    bf = lambda a: np.ascontiguousarray(np.asarray(a, np.float32).T).astype(
        ml_dtypes.bfloat16)
    f8 = lambda a: np.clip(np.ascontiguousarray(np.asarray(a, np.float32).T) * WS,
                           -240, 240).astype(ml_dtypes.float8_e4m3fn)
    f84 = lambda a: np.clip(np.ascontiguousarray(np.asarray(a, np.float32).T) * 4,
                            -240, 240).astype(ml_dtypes.float8_e4m3fn)
    qkv_w = np.asarray(inputs["qkv_w"], np.float32)
    tqkv_w = np.asarray(inputs["tqkv_w"], np.float32)
    mask, ident, sel, one8 = make_consts()
    return {
        "w_qk_t": f8(tqkv_w[:2 * C]), "w_v_t": f8(tqkv_w[2 * C:]),
        "w_qk_s": f8(qkv_w[:2 * C]), "w_v_s": f8(qkv_w[2 * C:]),
        "w_proj_t": f8(inputs["tproj_w"]),
        "w_tfc": f84(inputs["tfc_w"]),
        "w_proj_s": f8(inputs["proj_w"]),
        "w_fc1": bf(inputs["fc1_w"]), "w_fc2": bf(inputs["fc2_w"]),
        "mask_bd": mask.astype(ml_dtypes.bfloat16),
        "ident": ident.astype(ml_dtypes.bfloat16),
        "sel12": sel.astype(ml_dtypes.bfloat16),
        "one8": one8,
    }


def kernel(**inputs):
    x = np.ascontiguousarray(np.asarray(inputs["x"], dtype=np.float32))
    B = x.shape[0]
    shared = host_inputs(inputs)
    nc = _get_nc()
    in_maps = [dict(shared, x=np.ascontiguousarray(x[b])) for b in range(B)]
    res = run_bass_kernel_spmd(nc, in_maps, core_ids=list(range(B)),
                               trace=bool(int(os.environ.get("KTRACE", "0"))))
    out = np.stack([res.results[b]["out"] for b in range(B)], axis=0)
    kernel.last_results = res
    return out


# revision 6
# speedup vs baseline: 1.7372x; 1.7372x over previous
"""TimeSformer block (temporal attn -> spatial attn -> MLP) on 8 trn2 cores, v2.

Data-parallel over B=8: each NeuronCore processes one batch element.
v2: all matmul operands bf16 (FWL weight loads), qk/v/oT images SBUF-resident,
temporal attention as K=64 head-slice matmuls + post-exp block-diag mask,
spatial attention per-frame, MLP blocked with SBUF h1. Residual stream fp32.
"""

import os
import sys
from contextlib import ExitStack

sys.path.insert(0, "/opt/trn_rl_repo")

import numpy as np
import ml_dtypes

import concourse.bass as bass
import concourse.mybir as mybir
import concourse.tile as tile
from concourse import bacc
from concourse.bass_utils import run_bass_kernel_spmd

F32 = mybir.dt.float32
BF16 = mybir.dt.bfloat16
FP8 = mybir.dt.float8e4
DR = mybir.MatmulPerfMode.DoubleRow
WS = 16.0      # fp8 weight pre-scale; absorbed via sel12 (attn out) or eviction
AF = mybir.ActivationFunctionType
ALU = mybir.AluOpType

C = 768
CO = 6           # C / 128
H = 12
D = 64
T = 8
G = 196          # h*w sequences
NT = G * T       # 1568 temporal tokens
NSEQ = 197       # spatial seq len (cls + 196)
NS = 8 * NSEQ    # 1576 spatial tokens
N = 1569
HID = 3072
HIDO = 24        # HID / 128
P = 128
NSP = 1584       # NS padded so fp8 image chunk stride is 16B-aligned
EPS = 1e-5
SCALE = D ** -0.5

PHASES = os.environ.get("KPHASES", "12345678")
KDEBUG = bool(int(os.environ.get("KDEBUG", "0")))


def _ceil(a, b):
    return (a + b - 1) // b


def ln_tile_bf16(nc, tmp, out_pool, x_t, rows, eps_t):
    """LayerNorm over free dim (768) of a [rows<=128, 768] f32 tile -> bf16."""
    stats = tmp.tile([P, 2, 6], F32, tag="ln_stats")
    for s in range(2):
        nc.vector.bn_stats(out=stats[:rows, s], in_=x_t[:rows, s * 384:(s + 1) * 384])
    mv = tmp.tile([P, 2], F32, tag="ln_mv")
    nc.vector.bn_aggr(out=mv[:rows], in_=stats[:rows])
    nc.scalar.activation(out=mv[:rows, 1:2], in_=mv[:rows, 1:2], func=AF.Sqrt,
                         bias=eps_t[:rows], scale=1.0)
    nc.vector.reciprocal(out=mv[:rows, 1:2], in_=mv[:rows, 1:2])
    xln = out_pool.tile([P, C], BF16, tag="ln_out")
    nc.vector.tensor_scalar(out=xln[:rows], in0=x_t[:rows],
                            scalar1=mv[:rows, 0:1], scalar2=mv[:rows, 1:2],
                            op0=ALU.subtract, op1=ALU.mult)
    return xln


def qkv_common(nc, tc, ctx, src_rows_dma, ntok, w_qk, w_v, qk_img, v_img,
               v_tiles, ident, eps_t, xlnT):
    """LN -> transpose -> xlnT (c-major bf16); qk c-major image; v token-major
    per-head-padded image (ones in col 64 of each head's 65-wide slot).

    v_tiles: list of (row0, rows, tslot) token tiles for the v matmul/eviction.
    """
    wpool = ctx.enter_context(tc.tile_pool(name="w_qkv", bufs=1))
    lnp = ctx.enter_context(tc.tile_pool(name="lnp", bufs=4))
    lnout = ctx.enter_context(tc.tile_pool(name="lnout", bufs=4))
    tpp = ctx.enter_context(tc.tile_pool(name="tp_psum", bufs=2, space="PSUM"))
    mmp = ctx.enter_context(tc.tile_pool(name="qkv_psum", bufs=4, space="PSUM"))

    wqk = wpool.tile([P, CO, 2 * C], FP8)
    nc.gpsimd.dma_start(wqk, w_qk[:].rearrange("(co p) o -> p co o", p=P))
    wv = wpool.tile([P, CO, C], FP8)
    nc.gpsimd.dma_start(wv, w_v[:].rearrange("(co p) o -> p co o", p=P))

    ntiles = _ceil(ntok, P)
    for i in range(ntiles):
        rows = min(P, ntok - i * P)
        x_t = lnp.tile([P, C], F32, tag="x_t")
        src_rows_dma(x_t, i, rows)
        xln = ln_tile_bf16(nc, lnp, lnout, x_t, rows, eps_t)
        ptg = tpp.tile([P, CO, P], BF16, tag="tp_psum")
        for co in range(CO):
            nc.tensor.matmul(ptg[:, co, :rows], xln[:rows, co * P:(co + 1) * P],
                             ident[:rows, :rows], is_transpose=True,
                             start=(co == 0), stop=(co == CO - 1))
        nc.vector.tensor_copy(out=xlnT[:, :, i * P:i * P + rows],
                              in_=ptg[:, :, :rows])

    # v token-major per-head-padded (first: attention needs all v tiles)
    for tslot, rows, stat_fn in v_tiles:
        for half, fcols in ((0, 512), (1, 256)):
            ps = mmp.tile([P, 512], F32, tag="mm_ps")
            for ch in range(3):
                # stationary = xlnT chunk [c,2,tok], moving = wv [c,2,out]
                nc.tensor.matmul(
                    ps[:rows, :fcols], stat_fn(ch),
                    wv[:, 2 * ch:2 * ch + 2, half * 512:half * 512 + fcols],
                    start=(ch == 0), stop=(ch == 2), perf_mode=DR)
            # scatter head chunks into 65-strided per-head slots
            h0 = half * 8
            nc.scalar.activation(
                out=v_img[:rows, tslot, h0:h0 + fcols // D, 0:D],
                in_=ps[:rows, :fcols].rearrange("p (h d) -> p h d", d=D),
                func=AF.Copy, scale=1.0 / WS)
    # q,k c-major: image chunks 0..5 = q, 6..11 = k  (fp8 DoubleRow, K=256/chain)
    # o-order interleaved (q_h, k_h pairs) so attention head h unblocks early
    for o in [0, 6, 1, 7, 2, 8, 3, 9, 4, 10, 5, 11]:
        for b in range(_ceil(ntok, 512)):
            cols = min(512, ntok - b * 512)
            ps = mmp.tile([P, 512], F32, tag="mm_ps")
            for ch in range(3):
                nc.tensor.matmul(ps[:, :cols],
                                 wqk[:, 2 * ch:2 * ch + 2, o * P:(o + 1) * P],
                                 xlnT[:, 2 * ch:2 * ch + 2, b * 512:b * 512 + cols],
                                 start=(ch == 0), stop=(ch == 2), perf_mode=DR)
            nc.scalar.activation(out=qk_img[:, o, b * 512:b * 512 + cols],
                                 in_=ps[:, :cols], func=AF.Copy, scale=1.0 / WS)


def temporal_attn(nc, tc, qk_img, v_img, mask_bd, sel_sb, oT_img):
    with ExitStack() as ctx:
        sp = ctx.enter_context(tc.tile_pool(name="t_spsum", bufs=3, space="PSUM"))
        op = ctx.enter_context(tc.tile_pool(name="t_opsum", bufs=2, space="PSUM"))
        pp = ctx.enter_context(tc.tile_pool(name="t_p", bufs=6))
        sgp = ctx.enter_context(tc.tile_pool(name="t_sg", bufs=2))
        sig = ctx.enter_context(tc.tile_pool(name="t_sig", bufs=1))
        bcp = ctx.enter_context(tc.tile_pool(name="t_bc", bufs=2, space="PSUM"))

        sigma = sig.tile([12, NT], F32)
        rinv = sig.tile([12, NT], BF16)
        nst = _ceil(NT, P)  # 13 subtiles, grouped 4 per PSUM bank
        groups = []
        st = 0
        while st < nst:
            take = min(4, nst - st)
            g = [(s, min(P, NT - s * P)) for s in range(st, st + take)]
            groups.append((st * P, g, (take - 1) * P + g[-1][1]))
            st += take
        for h in range(H):
            hp = (h % 2) * D
            ch = h // 2
            sg_h = sgp.tile([1, NT], F32, tag="sg_h")
            for col0, g, cols in groups:
                ps = sp.tile([P, 512], F32, tag="s_ps")
                for j, (st_, rows) in enumerate(g):
                    sl0 = st_ * P
                    nc.tensor.matmul(ps[:rows, j * P:j * P + rows],
                                     qk_img[hp:hp + D, 6 + ch, sl0:sl0 + rows],
                                     qk_img[hp:hp + D, ch, sl0:sl0 + rows],
                                     start=(j == 0), stop=(j == len(g) - 1))
                rmax = g[0][1]
                p_t = pp.tile([P, 512], BF16, tag="p_t")
                nc.scalar.activation(out=p_t[:rmax, :cols], in_=ps[:rmax, :cols],
                                     func=AF.Exp, scale=SCALE)
                nc.gpsimd.tensor_mul(out=p_t[:rmax, :cols], in0=p_t[:rmax, :cols],
                                      in1=mask_bd[:rmax, :cols])
                po = op.tile([D + 1, 512], F32, tag="o_ps")
                for j, (st_, rows) in enumerate(g):
                    nc.tensor.matmul(po[:, j * P:j * P + rows],
                                     v_img[:rows, st_, h, :],
                                     p_t[:rows, j * P:j * P + rows],
                                     start=(j == 0), stop=(j == len(g) - 1))
                nc.vector.tensor_copy(out=oT_img[hp:hp + D, ch, col0:col0 + cols],
                                      in_=po[0:D, :cols])
                nc.vector.tensor_copy(out=sg_h[:, col0:col0 + cols],
                                      in_=po[D:D + 1, :cols])
            nc.sync.dma_start(sigma[h:h + 1], sg_h)
        with nc.allow_low_precision(reason="rinv feeds bf16 bcast matmul"):
            nc.vector.reciprocal(out=rinv, in_=sigma)
        for pr in range(6):
            for b in range(_ceil(NT, 512)):
                cols = min(512, NT - b * 512)
                bc = bcp.tile([P, 512], F32, tag="bc_ps")
                nc.tensor.matmul(bc[:, :cols], sel_sb[:, pr * P:(pr + 1) * P],
                                 rinv[:, b * 512:b * 512 + cols],
                                 start=True, stop=True)
                nc.vector.tensor_mul(
                    out=oT_img[:, pr, b * 512:b * 512 + cols],
                    in0=oT_img[:, pr, b * 512:b * 512 + cols],
                    in1=bc[:, :cols])


def temporal_proj(nc, tc, x_in, w_proj_t, w_tfc, oT_img, projT, xs_d):
    with ExitStack() as ctx:
        wp = ctx.enter_context(tc.tile_pool(name="p3_w", bufs=1))
        mp = ctx.enter_context(tc.tile_pool(name="p3_ps", bufs=4, space="PSUM"))
        tp = ctx.enter_context(tc.tile_pool(name="p3_t", bufs=3))

        wproj = wp.tile([P, CO, C], FP8)
        nc.sync.dma_start(wproj, w_proj_t[:].rearrange("(co p) o -> p co o", p=P))
        wtfc = wp.tile([P, CO, C], FP8)
        nc.sync.dma_start(wtfc, w_tfc[:].rearrange("(co p) o -> p co o", p=P))

        # xs_g rows 0..7 = x[0] (cls) for every frame; rows 8.. = xt (g-major)
        cls_sb = tp.tile([8, C], F32, tag="cls_sb")
        nc.gpsimd.dma_start(cls_sb, bass.AP(tensor=x_in, offset=0,
                                            ap=[[0, 8], [1, C]]))
        nc.sync.dma_start(xs_d[0:8, :], cls_sb)

        # projT = w_proj.T @ oT (c-major)
        for b in range(_ceil(NT, 512)):
            cols = min(512, NT - b * 512)
            for o in range(CO):
                ps = mp.tile([P, 512], F32, tag="p3ps")
                for ch in range(3):
                    nc.tensor.matmul(ps[:, :cols],
                                     wproj[:, 2 * ch:2 * ch + 2, o * P:(o + 1) * P],
                                     oT_img[:, 2 * ch:2 * ch + 2, b * 512:b * 512 + cols],
                                     start=(ch == 0), stop=(ch == 2), perf_mode=DR)
                nc.scalar.activation(out=projT[:, o, b * 512:b * 512 + cols],
                                     in_=ps[:, :cols], func=AF.Copy, scale=0.25)
        # xt = x + projT.T @ w_tfc, scattered to xs_d (t g c)
        for it in range(_ceil(NT, P)):
            tok0 = it * P
            rows = min(P, NT - tok0)
            x_t = tp.tile([P, C], F32, tag="x_t3")
            nc.sync.dma_start(x_t[:rows], x_in[1 + tok0:1 + tok0 + rows, :])
            xt = tp.tile([P, C], F32, tag="xt3")
            for half, fcols in ((0, 512), (1, 256)):
                ps = mp.tile([P, 512], F32, tag="p3ps")
                for ch in range(3):
                    nc.tensor.matmul(
                        ps[:rows, :fcols],
                        projT[:, 2 * ch:2 * ch + 2, tok0:tok0 + rows],
                        wtfc[:, 2 * ch:2 * ch + 2, half * 512:half * 512 + fcols],
                        start=(ch == 0), stop=(ch == 2), perf_mode=DR)
                nc.vector.tensor_add(
                    out=xt[:rows, half * 512:half * 512 + fcols],
                    in0=ps[:rows, :fcols],
                    in1=x_t[:rows, half * 512:half * 512 + fcols])
            nc.gpsimd.dma_start(xs_d[8 + tok0:8 + tok0 + rows, :], xt[:rows])


def spatial_attn(nc, tc, qk_img, v_img, sel_sb, oT_img):
    """g-major spatial attention: frame f = columns f::8 of the (j f) layout."""
    with ExitStack() as ctx:
        sp = ctx.enter_context(tc.tile_pool(name="s_spsum", bufs=2, space="PSUM"))
        op = ctx.enter_context(tc.tile_pool(name="s_opsum", bufs=2, space="PSUM"))
        pp = ctx.enter_context(tc.tile_pool(name="s_p", bufs=6))
        sgp = ctx.enter_context(tc.tile_pool(name="s_sg", bufs=2))
        sig = ctx.enter_context(tc.tile_pool(name="s_sig", bufs=1))
        bcp = ctx.enter_context(tc.tile_pool(name="s_bc", bufs=2, space="PSUM"))

        qk_r = qk_img[:].rearrange("p o (j f) -> p o j f", f=8)
        oT_r = oT_img[:].rearrange("p o (j f) -> p o j f", f=8)  # j=198 padded
        sigma = sig.tile([12, NS], F32)
        rinv = sig.tile([12, NS], BF16)
        CH1 = NSEQ - P  # 69
        for h in range(H):
            hp = (h % 2) * D
            ch = h // 2
            sg_h = sgp.tile([1, NS], F32, tag="sg_hs")
            sg_r = sg_h[:].rearrange("p (j f) -> p j f", f=8)
            for fp in range(4):
                f0 = 2 * fp
                ps0 = sp.tile([P, 2, NSEQ], F32, tag="s_ps0")
                ps1 = sp.tile([P, 2, NSEQ], F32, tag="s_ps1")
                for fi in range(2):
                    f = f0 + fi
                    q_sl = qk_r[hp:hp + D, ch, :, f]
                    nc.tensor.matmul(
                        ps0[:, fi], qk_r[hp:hp + D, 6 + ch, 0:P, f], q_sl,
                        start=(fi == 0), stop=(fi == 1))
                    nc.tensor.matmul(
                        ps1[:CH1, fi], qk_r[hp:hp + D, 6 + ch, P:NSEQ, f],
                        q_sl, start=(fi == 0), stop=(fi == 1))
                p0 = pp.tile([P, 2, NSEQ], BF16, tag="p_s0")
                p1 = pp.tile([P, 2, NSEQ], BF16, tag="p_s1")
                nc.scalar.activation(out=p0, in_=ps0, func=AF.Exp, scale=SCALE)
                nc.scalar.activation(out=p1[:CH1], in_=ps1[:CH1],
                                     func=AF.Exp, scale=SCALE)
                po = op.tile([D + 1, 2, NSEQ], F32, tag="o_ps_s")
                mms = [(fi, ci) for fi in range(2) for ci in range(2)]
                for idx, (fi, ci) in enumerate(mms):
                    f = f0 + fi
                    chlen = P if ci == 0 else CH1
                    psrc = p0 if ci == 0 else p1
                    nc.tensor.matmul(po[:, fi], v_img[:chlen, 2 * f + ci, h, :],
                                     psrc[:chlen, fi],
                                     start=(idx == 0), stop=(idx == len(mms) - 1))
                nc.vector.tensor_copy(
                    out=oT_r[hp:hp + D, ch, 0:NSEQ, f0:f0 + 2],
                    in_=po[0:D].rearrange("p f j -> p j f"))
                nc.vector.tensor_copy(out=sg_r[:, :, f0:f0 + 2],
                                      in_=po[D:D + 1].rearrange("p f j -> p j f"))
            nc.sync.dma_start(sigma[h:h + 1], sg_h)
        with nc.allow_low_precision(reason="rinv feeds bf16 bcast matmul"):
            nc.vector.reciprocal(out=rinv, in_=sigma)
        for pr in range(6):
            for b in range(_ceil(NS, 512)):
                cols = min(512, NS - b * 512)
                bc = bcp.tile([P, 512], F32, tag="bc_s")
                nc.tensor.matmul(bc[:, :cols], sel_sb[:, pr * P:(pr + 1) * P],
                                 rinv[:, b * 512:b * 512 + cols],
                                 start=True, stop=True)
                nc.vector.tensor_mul(
                    out=oT_img[:, pr, b * 512:b * 512 + cols],
                    in0=oT_img[:, pr, b * 512:b * 512 + cols],
                    in1=bc[:, :cols])


def spatial_proj(nc, tc, xs_d, w_proj_s, oT_img, y_s):
    with ExitStack() as ctx:
        wp = ctx.enter_context(tc.tile_pool(name="p6_w", bufs=1))
        mp = ctx.enter_context(tc.tile_pool(name="p6_ps", bufs=4, space="PSUM"))
        tp = ctx.enter_context(tc.tile_pool(name="p6_t", bufs=3))
        wproj = wp.tile([P, CO, C], FP8)
        nc.sync.dma_start(wproj, w_proj_s[:].rearrange("(co p) o -> p co o", p=P))
        for i in range(_ceil(NS, P)):
            rows = min(P, NS - i * P)
            x_t = tp.tile([P, C], F32, tag="x_t6")
            nc.gpsimd.dma_start(x_t[:rows], xs_d[i * P:i * P + rows, :])
            yt = tp.tile([P, C], F32, tag="yt6")
            for half, fcols in ((0, 512), (1, 256)):
                ps = mp.tile([P, 512], F32, tag="p6ps")
                for ch in range(3):
                    nc.tensor.matmul(
                        ps[:rows, :fcols],
                        oT_img[:, 2 * ch:2 * ch + 2, i * P:i * P + rows],
                        wproj[:, 2 * ch:2 * ch + 2, half * 512:half * 512 + fcols],
                        start=(ch == 0), stop=(ch == 2), perf_mode=DR)
                nc.vector.tensor_add(
                    out=yt[:rows, half * 512:half * 512 + fcols],
                    in0=ps[:rows, :fcols],
                    in1=x_t[:rows, half * 512:half * 512 + fcols])
            nc.gpsimd.dma_start(y_s[i * P:i * P + rows, :], yt[:rows])


def cls_mean(nc, tc, y_s, one8, cls_row):
    with ExitStack() as ctx:
        tp = ctx.enter_context(tc.tile_pool(name="p7_t", bufs=1))
        mp = ctx.enter_context(tc.tile_pool(name="p7_ps", bufs=2, space="PSUM"))
        ycls = tp.tile([8, C], F32)
        nc.sync.dma_start(ycls, y_s[0:8, :])
        o8 = tp.tile([8, 1], F32)
        nc.sync.dma_start(o8, one8[:])
        res = tp.tile([1, C], F32)
        for half, fcols in ((0, 512), (1, 256)):
            ps = mp.tile([1, 512], F32, tag="p7_ps")
            nc.tensor.matmul(ps[:, :fcols], o8,
                             ycls[:, half * 512:half * 512 + fcols],
                             start=True, stop=True)
            nc.vector.tensor_copy(out=res[:, half * 512:half * 512 + fcols],
                                  in_=ps[:, :fcols])
        nc.sync.dma_start(cls_row[:], res)


def mlp(nc, tc, y_s, cls_row, wfc1, wfc2, out, ident, eps_t):
    blocks = [(0, 512), (512, 512), (1024, 512), (1536, 33)]

    def load_x(pool, tok0, it, btok):
        rows = min(P, btok - it * P)
        x_t = pool.tile([P, C], F32, tag="x_t8")
        r0 = 8 + tok0 + it * P
        if btok == 33:
            nc.gpsimd.dma_start(x_t[:32], y_s[r0:r0 + 32, :])
            nc.gpsimd.dma_start(x_t[32:33], cls_row[:])
        else:
            nc.gpsimd.dma_start(x_t[:rows], y_s[r0:r0 + rows, :])
        return x_t, rows

    with ExitStack() as ctx:
        lnp = ctx.enter_context(tc.tile_pool(name="p8_ln", bufs=3))
        lnout = ctx.enter_context(tc.tile_pool(name="p8_lno", bufs=3))
        tpp = ctx.enter_context(tc.tile_pool(name="p8_tp", bufs=2, space="PSUM"))
        xlp = ctx.enter_context(tc.tile_pool(name="p8_xlT", bufs=2))
        m1p = ctx.enter_context(tc.tile_pool(name="p8_ps1", bufs=2, space="PSUM"))
        h1p = ctx.enter_context(tc.tile_pool(name="p8_h1", bufs=2))
        m2p = ctx.enter_context(tc.tile_pool(name="p8_ps2", bufs=2, space="PSUM"))
        xrp = ctx.enter_context(tc.tile_pool(name="p8_xr", bufs=5))
        otp = ctx.enter_context(tc.tile_pool(name="p8_o", bufs=3))

        for tok0, btok in blocks:
            btok_mm = btok + (btok % 2)
            xlnT = xlp.tile([P, CO, 512], BF16, tag="xlnT8")
            if btok % 2:
                nc.vector.memset(xlnT.bitcast(F32), 0.0)
            xts = []
            for it in range(_ceil(btok, P)):
                x_t, rows = load_x(xrp, tok0, it, btok)
                xts.append((x_t, rows))
                xln = ln_tile_bf16(nc, lnp, lnout, x_t, rows, eps_t)
                ptg = tpp.tile([P, CO, P], BF16, tag="tp8")
                for co in range(CO):
                    nc.tensor.matmul(ptg[:, co, :rows],
                                     xln[:rows, co * P:(co + 1) * P],
                                     ident[:rows, :rows], is_transpose=True,
                                     start=(co == 0), stop=(co == CO - 1))
                nc.vector.tensor_copy(out=xlnT[:, :, it * P:it * P + rows],
                                      in_=ptg[:, :, :rows])
            # fc1 + gelu -> h1 c-major bf16
            h1t = h1p.tile([P, HIDO, 512], BF16, tag="h1t")
            for o in range(HIDO):
                ps = m1p.tile([P, 512], F32, tag="p8ps1")
                for co in range(CO):
                    nc.tensor.matmul(ps[:, :btok_mm], wfc1[:, co, o * P:(o + 1) * P],
                                     xlnT[:, co, :btok_mm],
                                     start=(co == 0), stop=(co == CO - 1))
                nc.scalar.activation(out=h1t[:, o, :btok], in_=ps[:, :btok],
                                     func=AF.Gelu)
            # fc2 token-major + residual
            for it in range(_ceil(btok, P)):
                x_t, rows = xts[it]
                o_t = otp.tile([P, C], F32, tag="o_t8")
                for half, fcols in ((0, 512), (1, 256)):
                    ps = m2p.tile([P, 512], F32, tag="p8ps2")
                    for ho in range(HIDO):
                        nc.tensor.matmul(
                            ps[:rows, :fcols], h1t[:, ho, it * P:it * P + rows],
                            wfc2[:, ho, half * 512:half * 512 + fcols],
                            start=(ho == 0), stop=(ho == HIDO - 1))
                    nc.vector.tensor_add(
                        out=o_t[:rows, half * 512:half * 512 + fcols],
                        in0=ps[:rows, :fcols],
                        in1=x_t[:rows, half * 512:half * 512 + fcols])
                row0 = tok0 + it * P
                if btok == 33:
                    nc.gpsimd.dma_start(out[1 + row0:1 + row0 + 32, :], o_t[:32])
                    nc.gpsimd.dma_start(out[0:1, :], o_t[32:33])
                else:
                    nc.gpsimd.dma_start(out[1 + row0:1 + row0 + rows, :],
                                        o_t[:rows])


def build_nc():
    nc = bacc.Bacc("TRN2", target_bir_lowering=False, debug=False)

    x_in = nc.dram_tensor("x", (N, C), F32, kind="ExternalInput")
    w_qk_t = nc.dram_tensor("w_qk_t", (C, 2 * C), FP8, kind="ExternalInput")
    w_v_t = nc.dram_tensor("w_v_t", (C, C), FP8, kind="ExternalInput")
    w_qk_s = nc.dram_tensor("w_qk_s", (C, 2 * C), FP8, kind="ExternalInput")
    w_v_s = nc.dram_tensor("w_v_s", (C, C), FP8, kind="ExternalInput")
    w_proj_t = nc.dram_tensor("w_proj_t", (C, C), FP8, kind="ExternalInput")
    w_tfc = nc.dram_tensor("w_tfc", (C, C), FP8, kind="ExternalInput")
    w_proj_s = nc.dram_tensor("w_proj_s", (C, C), FP8, kind="ExternalInput")
    w_fc1 = nc.dram_tensor("w_fc1", (C, HID), BF16, kind="ExternalInput")
    w_fc2 = nc.dram_tensor("w_fc2", (HID, C), BF16, kind="ExternalInput")
    mask_in = nc.dram_tensor("mask_bd", (P, 512), BF16, kind="ExternalInput")
    ident_in = nc.dram_tensor("ident", (P, P), BF16, kind="ExternalInput")
    sel12 = nc.dram_tensor("sel12", (12, C), BF16, kind="ExternalInput")
    one8 = nc.dram_tensor("one8", (8, 1), F32, kind="ExternalInput")
    out = nc.dram_tensor("out", (N, C), F32, kind="ExternalOutput")
    dbg = {}
    if KDEBUG:
        for nm, shp, dt_ in (("d_qk_t", (P, 12, NT), BF16),
                             ("d_v_t", (P, 16, 12, D + 1), BF16),
                             ("d_oT_t", (P, CO, NT), FP8),
                             ("d_xs", (NS, C), F32),
                             ("d_qk_s", (P, 12, NS), BF16),
                             ("d_oT_s", (P, CO, NS), FP8),
                             ("d_y_s", (NS, C), F32),
                             ("d_cls", (1, C), F32)):
            dbg[nm] = nc.dram_tensor(nm, shp, dt_, kind="ExternalOutput")

    # v tile specs are built inside build (need xlnT handle for stationary fns)

    with tile.TileContext(nc) as tc:
        with tc.tile_pool(name="dram", bufs=1, space="DRAM") as dram, \
             tc.tile_pool(name="const", bufs=1) as const:
            xs_d = dram.tile([NS, C], F32)
            y_s = dram.tile([NS, C], F32)
            cls_row = dram.tile([1, C], F32)

            ident = const.tile([P, P], BF16)
            nc.sync.dma_start(ident, ident_in[:])
            mask_bd = const.tile([P, 512], BF16)
            nc.sync.dma_start(mask_bd, mask_in[:])
            eps_t = const.tile([P, 1], F32)
            nc.vector.memset(eps_t, EPS)
            sel_sb = const.tile([12, C], BF16)
            nc.sync.dma_start(sel_sb, sel12[:])

            wfc1 = const.tile([P, CO, HID], BF16)
            wfc2 = const.tile([P, HIDO, C], BF16)
            nc.gpsimd.dma_start(wfc1, w_fc1[:].rearrange("(co p) o -> p co o", p=P))
            nc.gpsimd.dma_start(wfc2, w_fc2[:].rearrange("(ho p) o -> p ho o", p=P))

            img_ctx = ExitStack()
            img = img_ctx.enter_context(tc.tile_pool(name="img", bufs=1))
            qk_img = img.tile([P, 12, NS], BF16)
            v_img = img.tile([P, 16, 12, D + 1], BF16)
            oT_img = img.tile([P, CO, NSP], FP8)
            xlnT = img.tile([P, CO, NSP], FP8)
            projT = img.tile([P, CO, NT], FP8)

            # ones column in every v slot (col D of each head slot)
            if KDEBUG:
                nc.vector.memset(v_img, 1.0)
            else:
                nc.vector.memset(v_img[:, :, :, D:D + 1], 1.0)

            def mk_stat(row0, rows):
                return lambda ch: xlnT[:, 2 * ch:2 * ch + 2, row0:row0 + rows]

            xlnT_r = xlnT[:].rearrange("p o (j f) -> p o j f", f=8)  # j=198 padded

            def mk_stat_f(f, c0, chlen):
                return lambda ch: xlnT_r[:, 2 * ch:2 * ch + 2, c0:c0 + chlen, f]

            v_tiles_t = [(i, min(P, NT - i * P), mk_stat(i * P, min(P, NT - i * P)))
                         for i in range(_ceil(NT, P))]
            v_tiles_s = []
            for f in range(8):
                v_tiles_s.append((2 * f, P, mk_stat_f(f, 0, P)))
                v_tiles_s.append((2 * f + 1, NSEQ - P, mk_stat_f(f, P, NSEQ - P)))

            if "1" in PHASES:
                with ExitStack() as ctx:
                    def src_t(x_t, i, rows):
                        nc.sync.dma_start(x_t[:rows],
                                          x_in[1 + i * P:1 + i * P + rows, :])
                    qkv_common(nc, tc, ctx, src_t, NT, w_qk_t, w_v_t,
                               qk_img, v_img, v_tiles_t, ident, eps_t, xlnT)
            if "2" in PHASES:
                temporal_attn(nc, tc, qk_img, v_img, mask_bd, sel_sb, oT_img)
                if KDEBUG:
                    nc.sync.dma_start(dbg["d_oT_t"][:], oT_img[:, :, :NT])
            if KDEBUG and "1" in PHASES:
                nc.sync.dma_start(dbg["d_qk_t"][:], qk_img[:, :, :NT])
                nc.sync.dma_start(dbg["d_v_t"][:], v_img)
            if "3" in PHASES:
                temporal_proj(nc, tc, x_in, w_proj_t, w_tfc, oT_img, projT, xs_d)
                if KDEBUG:
                    nc.sync.dma_start(dbg["d_xs"][:], xs_d[:])
            if "4" in PHASES:
                with ExitStack() as ctx:
                    def src_s(x_t, i, rows):
                        nc.sync.dma_start(x_t[:rows],
                                          xs_d[i * P:i * P + rows, :])
                    qkv_common(nc, tc, ctx, src_s, NS, w_qk_s, w_v_s,
                               qk_img, v_img, v_tiles_s, ident, eps_t, xlnT)
                if KDEBUG:
                    nc.sync.dma_start(dbg["d_qk_s"][:], qk_img)
            if "5" in PHASES:
                spatial_attn(nc, tc, qk_img, v_img, sel_sb, oT_img)
                if KDEBUG:
                    nc.sync.dma_start(dbg["d_oT_s"][:], oT_img[:, :, :NS])
            if "6" in PHASES:
                spatial_proj(nc, tc, xs_d, w_proj_s, oT_img, y_s)
                if KDEBUG:
                    nc.sync.dma_start(dbg["d_y_s"][:], y_s[:])
            img_ctx.close()
            if "7" in PHASES:
                cls_mean(nc, tc, y_s, one8, cls_row)
                if KDEBUG:
                    nc.sync.dma_start(dbg["d_cls"][:], cls_row[:])
            if "8" in PHASES:
                mlp(nc, tc, y_s, cls_row, wfc1, wfc2, out, ident, eps_t)

    nc.compile()
    return nc


_NC_CACHE = None


def _get_nc():
    global _NC_CACHE
    if _NC_CACHE is None:
        _NC_CACHE = build_nc()
    return _NC_CACHE


def make_consts():
    idx = np.arange(P)
    mask = (idx[:, None] // T == idx[None, :] // T).astype(np.float32)
    mask = np.tile(mask, (1, 4))
    ident = np.eye(P, dtype=np.float32)
    sel = np.zeros((12, C), np.float32)
    for pr in range(6):
        for p in range(P):
            sel[2 * pr + p // D, pr * P + p] = 1.0 / WS
    one8 = np.full((8, 1), 0.125, np.float32)
    return mask, ident, sel, one8


def host_inputs(inputs):
    bf = lambda a: np.ascontiguousarray(np.asarray(a, np.float32).T).astype(
        ml_dtypes.bfloat16)
    f8 = lambda a: np.clip(np.ascontiguousarray(np.asarray(a, np.float32).T) * WS,
                           -240, 240).astype(ml_dtypes.float8_e4m3fn)
    f84 = lambda a: np.clip(np.ascontiguousarray(np.asarray(a, np.float32).T) * 4,
                            -240, 240).astype(ml_dtypes.float8_e4m3fn)
    qkv_w = np.asarray(inputs["qkv_w"], np.float32)
    tqkv_w = np.asarray(inputs["tqkv_w"], np.float32)
    mask, ident, sel, one8 = make_consts()
    return {
        "w_qk_t": f8(tqkv_w[:2 * C]), "w_v_t": f8(tqkv_w[2 * C:]),
        "w_qk_s": f8(qkv_w[:2 * C]), "w_v_s": f8(qkv_w[2 * C:]),
        "w_proj_t": f8(inputs["tproj_w"]),
        "w_tfc": f84(inputs["tfc_w"]),
        "w_proj_s": f8(inputs["proj_w"]),
        "w_fc1": bf(inputs["fc1_w"]), "w_fc2": bf(inputs["fc2_w"]),
        "mask_bd": mask.astype(ml_dtypes.bfloat16),
        "ident": ident.astype(ml_dtypes.bfloat16),
        "sel12": sel.astype(ml_dtypes.bfloat16),
        "one8": one8,
    }


def kernel(**inputs):
    x = np.ascontiguousarray(np.asarray(inputs["x"], dtype=np.float32))
    B = x.shape[0]
    shared = host_inputs(inputs)
    nc = _get_nc()
    in_maps = [dict(shared, x=np.ascontiguousarray(x[b])) for b in range(B)]
    res = run_bass_kernel_spmd(nc, in_maps, core_ids=list(range(B)),
                               trace=bool(int(os.environ.get("KTRACE", "0"))))
    out = np.stack([res.results[b]["out"] for b in range(B)], axis=0)
    kernel.last_results = res
    return out


# revision 7
# speedup vs baseline: 1.7401x; 1.0017x over previous
"""TimeSformer block (temporal attn -> spatial attn -> MLP) on 8 trn2 cores, v2.

Data-parallel over B=8: each NeuronCore processes one batch element.
v2: all matmul operands bf16 (FWL weight loads), qk/v/oT images SBUF-resident,
temporal attention as K=64 head-slice matmuls + post-exp block-diag mask,
spatial attention per-frame, MLP blocked with SBUF h1. Residual stream fp32.
"""

import os
import sys
from contextlib import ExitStack

sys.path.insert(0, "/opt/trn_rl_repo")

import numpy as np
import ml_dtypes

import concourse.bass as bass
import concourse.mybir as mybir
import concourse.tile as tile
from concourse import bacc
from concourse.bass_utils import run_bass_kernel_spmd

F32 = mybir.dt.float32
BF16 = mybir.dt.bfloat16
FP8 = mybir.dt.float8e4
DR = mybir.MatmulPerfMode.DoubleRow
WS = 16.0      # fp8 weight pre-scale; absorbed via sel12 (attn out) or eviction
AF = mybir.ActivationFunctionType
ALU = mybir.AluOpType

C = 768
CO = 6           # C / 128
H = 12
D = 64
T = 8
G = 196          # h*w sequences
NT = G * T       # 1568 temporal tokens
NSEQ = 197       # spatial seq len (cls + 196)
NS = 8 * NSEQ    # 1576 spatial tokens
N = 1569
HID = 3072
HIDO = 24        # HID / 128
P = 128
NSP = 1584       # NS padded so fp8 image chunk stride is 16B-aligned
EPS = 1e-5
SCALE = D ** -0.5

PHASES = os.environ.get("KPHASES", "12345678")
KDEBUG = bool(int(os.environ.get("KDEBUG", "0")))


def _ceil(a, b):
    return (a + b - 1) // b


def ln_tile_bf16(nc, tmp, out_pool, x_t, rows, eps_t):
    """LayerNorm over free dim (768) of a [rows<=128, 768] f32 tile -> bf16."""
    stats = tmp.tile([P, 2, 6], F32, tag="ln_stats")
    for s in range(2):
        nc.vector.bn_stats(out=stats[:rows, s], in_=x_t[:rows, s * 384:(s + 1) * 384])
    mv = tmp.tile([P, 2], F32, tag="ln_mv")
    nc.vector.bn_aggr(out=mv[:rows], in_=stats[:rows])
    nc.scalar.activation(out=mv[:rows, 1:2], in_=mv[:rows, 1:2], func=AF.Sqrt,
                         bias=eps_t[:rows], scale=1.0)
    nc.vector.reciprocal(out=mv[:rows, 1:2], in_=mv[:rows, 1:2])
    xln = out_pool.tile([P, C], BF16, tag="ln_out")
    nc.vector.tensor_scalar(out=xln[:rows], in0=x_t[:rows],
                            scalar1=mv[:rows, 0:1], scalar2=mv[:rows, 1:2],
                            op0=ALU.subtract, op1=ALU.mult)
    return xln


def qkv_common(nc, tc, ctx, src_rows_dma, ntok, w_qk, w_v, qk_img, v_img,
               v_tiles, ident, eps_t, xlnT):
    """LN -> transpose -> xlnT (c-major bf16); qk c-major image; v token-major
    per-head-padded image (ones in col 64 of each head's 65-wide slot).

    v_tiles: list of (row0, rows, tslot) token tiles for the v matmul/eviction.
    """
    wpool = ctx.enter_context(tc.tile_pool(name="w_qkv", bufs=1))
    lnp = ctx.enter_context(tc.tile_pool(name="lnp", bufs=4))
    lnout = ctx.enter_context(tc.tile_pool(name="lnout", bufs=4))
    tpp = ctx.enter_context(tc.tile_pool(name="tp_psum", bufs=2, space="PSUM"))
    mmp = ctx.enter_context(tc.tile_pool(name="qkv_psum", bufs=4, space="PSUM"))

    wqk = wpool.tile([P, CO, 2 * C], FP8)
    nc.gpsimd.dma_start(wqk, w_qk[:].rearrange("(co p) o -> p co o", p=P))
    wv = wpool.tile([P, CO, C], FP8)
    nc.gpsimd.dma_start(wv, w_v[:].rearrange("(co p) o -> p co o", p=P))

    ntiles = _ceil(ntok, P)
    for i in range(ntiles):
        rows = min(P, ntok - i * P)
        x_t = lnp.tile([P, C], F32, tag="x_t")
        src_rows_dma(x_t, i, rows)
        xln = ln_tile_bf16(nc, lnp, lnout, x_t, rows, eps_t)
        ptg = tpp.tile([P, CO, P], BF16, tag="tp_psum")
        for co in range(CO):
            nc.tensor.matmul(ptg[:, co, :rows], xln[:rows, co * P:(co + 1) * P],
                             ident[:rows, :rows], is_transpose=True,
                             start=(co == 0), stop=(co == CO - 1))
        nc.vector.tensor_copy(out=xlnT[:, :, i * P:i * P + rows],
                              in_=ptg[:, :, :rows])

    # v token-major per-head-padded (first: attention needs all v tiles)
    for tslot, rows, stat_fn in v_tiles:
        for half, fcols in ((0, 512), (1, 256)):
            ps = mmp.tile([P, 512], F32, tag="mm_ps")
            for ch in range(3):
                # stationary = xlnT chunk [c,2,tok], moving = wv [c,2,out]
                nc.tensor.matmul(
                    ps[:rows, :fcols], stat_fn(ch),
                    wv[:, 2 * ch:2 * ch + 2, half * 512:half * 512 + fcols],
                    start=(ch == 0), stop=(ch == 2), perf_mode=DR)
            # scatter head chunks into 65-strided per-head slots
            h0 = half * 8
            nc.scalar.activation(
                out=v_img[:rows, tslot, h0:h0 + fcols // D, 0:D],
                in_=ps[:rows, :fcols].rearrange("p (h d) -> p h d", d=D),
                func=AF.Copy, scale=1.0 / WS)
    # q,k c-major: image chunks 0..5 = q, 6..11 = k  (fp8 DoubleRow, K=256/chain)
    # o-order interleaved (q_h, k_h pairs) so attention head h unblocks early
    for o in [0, 6, 1, 7, 2, 8, 3, 9, 4, 10, 5, 11]:
        for b in range(_ceil(ntok, 512)):
            cols = min(512, ntok - b * 512)
            ps = mmp.tile([P, 512], F32, tag="mm_ps")
            for ch in range(3):
                nc.tensor.matmul(ps[:, :cols],
                                 wqk[:, 2 * ch:2 * ch + 2, o * P:(o + 1) * P],
                                 xlnT[:, 2 * ch:2 * ch + 2, b * 512:b * 512 + cols],
                                 start=(ch == 0), stop=(ch == 2), perf_mode=DR)
            nc.vector.tensor_scalar_mul(out=qk_img[:, o, b * 512:b * 512 + cols],
                                        in0=ps[:, :cols], scalar1=1.0 / WS)


def temporal_attn(nc, tc, qk_img, v_img, mask_bd, sel_sb, oT_img):
    with ExitStack() as ctx:
        sp = ctx.enter_context(tc.tile_pool(name="t_spsum", bufs=3, space="PSUM"))
        op = ctx.enter_context(tc.tile_pool(name="t_opsum", bufs=2, space="PSUM"))
        pp = ctx.enter_context(tc.tile_pool(name="t_p", bufs=6))
        sgp = ctx.enter_context(tc.tile_pool(name="t_sg", bufs=2))
        sig = ctx.enter_context(tc.tile_pool(name="t_sig", bufs=1))
        bcp = ctx.enter_context(tc.tile_pool(name="t_bc", bufs=2, space="PSUM"))

        sigma = sig.tile([12, NT], F32)
        rinv = sig.tile([12, NT], BF16)
        nst = _ceil(NT, P)  # 13 subtiles, grouped 4 per PSUM bank
        groups = []
        st = 0
        while st < nst:
            take = min(4, nst - st)
            g = [(s, min(P, NT - s * P)) for s in range(st, st + take)]
            groups.append((st * P, g, (take - 1) * P + g[-1][1]))
            st += take
        for h in range(H):
            hp = (h % 2) * D
            ch = h // 2
            sg_h = sgp.tile([1, NT], F32, tag="sg_h")
            for col0, g, cols in groups:
                ps = sp.tile([P, 512], F32, tag="s_ps")
                for j, (st_, rows) in enumerate(g):
                    sl0 = st_ * P
                    nc.tensor.matmul(ps[:rows, j * P:j * P + rows],
                                     qk_img[hp:hp + D, 6 + ch, sl0:sl0 + rows],
                                     qk_img[hp:hp + D, ch, sl0:sl0 + rows],
                                     start=(j == 0), stop=(j == len(g) - 1))
                rmax = g[0][1]
                p_t = pp.tile([P, 512], BF16, tag="p_t")
                nc.scalar.activation(out=p_t[:rmax, :cols], in_=ps[:rmax, :cols],
                                     func=AF.Exp, scale=SCALE)
                nc.gpsimd.tensor_mul(out=p_t[:rmax, :cols], in0=p_t[:rmax, :cols],
                                      in1=mask_bd[:rmax, :cols])
                po = op.tile([D + 1, 512], F32, tag="o_ps")
                for j, (st_, rows) in enumerate(g):
                    nc.tensor.matmul(po[:, j * P:j * P + rows],
                                     v_img[:rows, st_, h, :],
                                     p_t[:rows, j * P:j * P + rows],
                                     start=(j == 0), stop=(j == len(g) - 1))
                nc.vector.tensor_copy(out=oT_img[hp:hp + D, ch, col0:col0 + cols],
                                      in_=po[0:D, :cols])
                nc.vector.tensor_copy(out=sg_h[:, col0:col0 + cols],
                                      in_=po[D:D + 1, :cols])
            nc.sync.dma_start(sigma[h:h + 1], sg_h)
        with nc.allow_low_precision(reason="rinv feeds bf16 bcast matmul"):
            nc.vector.reciprocal(out=rinv, in_=sigma)
        for pr in range(6):
            for b in range(_ceil(NT, 512)):
                cols = min(512, NT - b * 512)
                bc = bcp.tile([P, 512], F32, tag="bc_ps")
                nc.tensor.matmul(bc[:, :cols], sel_sb[:, pr * P:(pr + 1) * P],
                                 rinv[:, b * 512:b * 512 + cols],
                                 start=True, stop=True)
                nc.vector.tensor_mul(
                    out=oT_img[:, pr, b * 512:b * 512 + cols],
                    in0=oT_img[:, pr, b * 512:b * 512 + cols],
                    in1=bc[:, :cols])


def temporal_proj(nc, tc, x_in, w_proj_t, w_tfc, oT_img, projT, xs_d):
    with ExitStack() as ctx:
        wp = ctx.enter_context(tc.tile_pool(name="p3_w", bufs=1))
        mp = ctx.enter_context(tc.tile_pool(name="p3_ps", bufs=4, space="PSUM"))
        tp = ctx.enter_context(tc.tile_pool(name="p3_t", bufs=3))

        wproj = wp.tile([P, CO, C], FP8)
        nc.sync.dma_start(wproj, w_proj_t[:].rearrange("(co p) o -> p co o", p=P))
        wtfc = wp.tile([P, CO, C], FP8)
        nc.sync.dma_start(wtfc, w_tfc[:].rearrange("(co p) o -> p co o", p=P))

        # xs_g rows 0..7 = x[0] (cls) for every frame; rows 8.. = xt (g-major)
        cls_sb = tp.tile([8, C], F32, tag="cls_sb")
        nc.gpsimd.dma_start(cls_sb, bass.AP(tensor=x_in, offset=0,
                                            ap=[[0, 8], [1, C]]))
        nc.sync.dma_start(xs_d[0:8, :], cls_sb)

        # projT = w_proj.T @ oT (c-major)
        for b in range(_ceil(NT, 512)):
            cols = min(512, NT - b * 512)
            for o in range(CO):
                ps = mp.tile([P, 512], F32, tag="p3ps")
                for ch in range(3):
                    nc.tensor.matmul(ps[:, :cols],
                                     wproj[:, 2 * ch:2 * ch + 2, o * P:(o + 1) * P],
                                     oT_img[:, 2 * ch:2 * ch + 2, b * 512:b * 512 + cols],
                                     start=(ch == 0), stop=(ch == 2), perf_mode=DR)
                nc.scalar.activation(out=projT[:, o, b * 512:b * 512 + cols],
                                     in_=ps[:, :cols], func=AF.Copy, scale=0.25)
        # xt = x + projT.T @ w_tfc, scattered to xs_d (t g c)
        for it in range(_ceil(NT, P)):
            tok0 = it * P
            rows = min(P, NT - tok0)
            x_t = tp.tile([P, C], F32, tag="x_t3")
            nc.sync.dma_start(x_t[:rows], x_in[1 + tok0:1 + tok0 + rows, :])
            xt = tp.tile([P, C], F32, tag="xt3")
            for half, fcols in ((0, 512), (1, 256)):
                ps = mp.tile([P, 512], F32, tag="p3ps")
                for ch in range(3):
                    nc.tensor.matmul(
                        ps[:rows, :fcols],
                        projT[:, 2 * ch:2 * ch + 2, tok0:tok0 + rows],
                        wtfc[:, 2 * ch:2 * ch + 2, half * 512:half * 512 + fcols],
                        start=(ch == 0), stop=(ch == 2), perf_mode=DR)
                nc.vector.tensor_add(
                    out=xt[:rows, half * 512:half * 512 + fcols],
                    in0=ps[:rows, :fcols],
                    in1=x_t[:rows, half * 512:half * 512 + fcols])
            nc.gpsimd.dma_start(xs_d[8 + tok0:8 + tok0 + rows, :], xt[:rows])


def spatial_attn(nc, tc, qk_img, v_img, sel_sb, oT_img):
    """g-major spatial attention: frame f = columns f::8 of the (j f) layout."""
    with ExitStack() as ctx:
        sp = ctx.enter_context(tc.tile_pool(name="s_spsum", bufs=2, space="PSUM"))
        op = ctx.enter_context(tc.tile_pool(name="s_opsum", bufs=2, space="PSUM"))
        pp = ctx.enter_context(tc.tile_pool(name="s_p", bufs=6))
        sgp = ctx.enter_context(tc.tile_pool(name="s_sg", bufs=2))
        sig = ctx.enter_context(tc.tile_pool(name="s_sig", bufs=1))
        bcp = ctx.enter_context(tc.tile_pool(name="s_bc", bufs=2, space="PSUM"))

        qk_r = qk_img[:].rearrange("p o (j f) -> p o j f", f=8)
        oT_r = oT_img[:].rearrange("p o (j f) -> p o j f", f=8)  # j=198 padded
        sigma = sig.tile([12, NS], F32)
        rinv = sig.tile([12, NS], BF16)
        CH1 = NSEQ - P  # 69
        for h in range(H):
            hp = (h % 2) * D
            ch = h // 2
            sg_h = sgp.tile([1, NS], F32, tag="sg_hs")
            sg_r = sg_h[:].rearrange("p (j f) -> p j f", f=8)
            for fp in range(4):
                f0 = 2 * fp
                ps0 = sp.tile([P, 2, NSEQ], F32, tag="s_ps0")
                ps1 = sp.tile([P, 2, NSEQ], F32, tag="s_ps1")
                for fi in range(2):
                    f = f0 + fi
                    q_sl = qk_r[hp:hp + D, ch, :, f]
                    nc.tensor.matmul(
                        ps0[:, fi], qk_r[hp:hp + D, 6 + ch, 0:P, f], q_sl,
                        start=(fi == 0), stop=(fi == 1))
                    nc.tensor.matmul(
                        ps1[:CH1, fi], qk_r[hp:hp + D, 6 + ch, P:NSEQ, f],
                        q_sl, start=(fi == 0), stop=(fi == 1))
                p0 = pp.tile([P, 2, NSEQ], BF16, tag="p_s0")
                p1 = pp.tile([P, 2, NSEQ], BF16, tag="p_s1")
                nc.scalar.activation(out=p0, in_=ps0, func=AF.Exp, scale=SCALE)
                nc.scalar.activation(out=p1[:CH1], in_=ps1[:CH1],
                                     func=AF.Exp, scale=SCALE)
                po = op.tile([D + 1, 2, NSEQ], F32, tag="o_ps_s")
                mms = [(fi, ci) for fi in range(2) for ci in range(2)]
                for idx, (fi, ci) in enumerate(mms):
                    f = f0 + fi
                    chlen = P if ci == 0 else CH1
                    psrc = p0 if ci == 0 else p1
                    nc.tensor.matmul(po[:, fi], v_img[:chlen, 2 * f + ci, h, :],
                                     psrc[:chlen, fi],
                                     start=(idx == 0), stop=(idx == len(mms) - 1))
                nc.vector.tensor_copy(
                    out=oT_r[hp:hp + D, ch, 0:NSEQ, f0:f0 + 2],
                    in_=po[0:D].rearrange("p f j -> p j f"))
                nc.vector.tensor_copy(out=sg_r[:, :, f0:f0 + 2],
                                      in_=po[D:D + 1].rearrange("p f j -> p j f"))
            nc.sync.dma_start(sigma[h:h + 1], sg_h)
        with nc.allow_low_precision(reason="rinv feeds bf16 bcast matmul"):
            nc.vector.reciprocal(out=rinv, in_=sigma)
        for pr in range(6):
            for b in range(_ceil(NS, 512)):
                cols = min(512, NS - b * 512)
                bc = bcp.tile([P, 512], F32, tag="bc_s")
                nc.tensor.matmul(bc[:, :cols], sel_sb[:, pr * P:(pr + 1) * P],
                                 rinv[:, b * 512:b * 512 + cols],
                                 start=True, stop=True)
                nc.vector.tensor_mul(
                    out=oT_img[:, pr, b * 512:b * 512 + cols],
                    in0=oT_img[:, pr, b * 512:b * 512 + cols],
                    in1=bc[:, :cols])


def spatial_proj(nc, tc, xs_d, w_proj_s, oT_img, y_s, ycls_sb):
    with ExitStack() as ctx:
        wp = ctx.enter_context(tc.tile_pool(name="p6_w", bufs=1))
        mp = ctx.enter_context(tc.tile_pool(name="p6_ps", bufs=4, space="PSUM"))
        tp = ctx.enter_context(tc.tile_pool(name="p6_t", bufs=3))
        wproj = wp.tile([P, CO, C], FP8)
        nc.sync.dma_start(wproj, w_proj_s[:].rearrange("(co p) o -> p co o", p=P))
        for i in range(_ceil(NS, P)):
            rows = min(P, NS - i * P)
            x_t = tp.tile([P, C], F32, tag="x_t6")
            nc.gpsimd.dma_start(x_t[:rows], xs_d[i * P:i * P + rows, :])
            yt = tp.tile([P, C], F32, tag="yt6")
            for half, fcols in ((0, 512), (1, 256)):
                ps = mp.tile([P, 512], F32, tag="p6ps")
                for ch in range(3):
                    nc.tensor.matmul(
                        ps[:rows, :fcols],
                        oT_img[:, 2 * ch:2 * ch + 2, i * P:i * P + rows],
                        wproj[:, 2 * ch:2 * ch + 2, half * 512:half * 512 + fcols],
                        start=(ch == 0), stop=(ch == 2), perf_mode=DR)
                nc.vector.tensor_add(
                    out=yt[:rows, half * 512:half * 512 + fcols],
                    in0=ps[:rows, :fcols],
                    in1=x_t[:rows, half * 512:half * 512 + fcols])
            if i == 0:
                nc.vector.tensor_copy(out=ycls_sb, in_=yt[0:8])
            nc.gpsimd.dma_start(y_s[i * P:i * P + rows, :], yt[:rows])


def cls_mean(nc, tc, ycls_sb, one8, cls_sb):
    with ExitStack() as ctx:
        tp = ctx.enter_context(tc.tile_pool(name="p7_t", bufs=1))
        mp = ctx.enter_context(tc.tile_pool(name="p7_ps", bufs=2, space="PSUM"))
        o8 = tp.tile([8, 1], F32)
        nc.sync.dma_start(o8, one8[:])
        for half, fcols in ((0, 512), (1, 256)):
            ps = mp.tile([1, 512], F32, tag="p7_ps")
            nc.tensor.matmul(ps[:, :fcols], o8,
                             ycls_sb[:, half * 512:half * 512 + fcols],
                             start=True, stop=True)
            nc.vector.tensor_copy(out=cls_sb[:, half * 512:half * 512 + fcols],
                                  in_=ps[:, :fcols])


def mlp(nc, tc, y_s, cls_row, wfc1, wfc2, out, ident, eps_t):
    blocks = [(0, 512), (512, 512), (1024, 512), (1536, 33)]

    def load_x(pool, tok0, it, btok):
        rows = min(P, btok - it * P)
        x_t = pool.tile([P, C], F32, tag="x_t8")
        r0 = 8 + tok0 + it * P
        if btok == 33:
            nc.gpsimd.dma_start(x_t[:32], y_s[r0:r0 + 32, :])
            nc.vector.tensor_copy(out=x_t[32:33], in_=cls_row[:])
        else:
            nc.gpsimd.dma_start(x_t[:rows], y_s[r0:r0 + rows, :])
        return x_t, rows

    with ExitStack() as ctx:
        lnp = ctx.enter_context(tc.tile_pool(name="p8_ln", bufs=3))
        lnout = ctx.enter_context(tc.tile_pool(name="p8_lno", bufs=3))
        tpp = ctx.enter_context(tc.tile_pool(name="p8_tp", bufs=2, space="PSUM"))
        xlp = ctx.enter_context(tc.tile_pool(name="p8_xlT", bufs=2))
        m1p = ctx.enter_context(tc.tile_pool(name="p8_ps1", bufs=2, space="PSUM"))
        h1p = ctx.enter_context(tc.tile_pool(name="p8_h1", bufs=2))
        m2p = ctx.enter_context(tc.tile_pool(name="p8_ps2", bufs=2, space="PSUM"))
        xrp = ctx.enter_context(tc.tile_pool(name="p8_xr", bufs=5))
        otp = ctx.enter_context(tc.tile_pool(name="p8_o", bufs=3))

        for tok0, btok in blocks:
            btok_mm = btok + (btok % 2)
            xlnT = xlp.tile([P, CO, 512], BF16, tag="xlnT8")
            if btok % 2:
                nc.vector.memset(xlnT.bitcast(F32), 0.0)
            xts = []
            for it in range(_ceil(btok, P)):
                x_t, rows = load_x(xrp, tok0, it, btok)
                xts.append((x_t, rows))
                xln = ln_tile_bf16(nc, lnp, lnout, x_t, rows, eps_t)
                ptg = tpp.tile([P, CO, P], BF16, tag="tp8")
                for co in range(CO):
                    nc.tensor.matmul(ptg[:, co, :rows],
                                     xln[:rows, co * P:(co + 1) * P],
                                     ident[:rows, :rows], is_transpose=True,
                                     start=(co == 0), stop=(co == CO - 1))
                nc.vector.tensor_copy(out=xlnT[:, :, it * P:it * P + rows],
                                      in_=ptg[:, :, :rows])
            # fc1 + gelu -> h1 c-major bf16
            h1t = h1p.tile([P, HIDO, 512], BF16, tag="h1t")
            for o in range(HIDO):
                ps = m1p.tile([P, 512], F32, tag="p8ps1")
                for co in range(CO):
                    nc.tensor.matmul(ps[:, :btok_mm], wfc1[:, co, o * P:(o + 1) * P],
                                     xlnT[:, co, :btok_mm],
                                     start=(co == 0), stop=(co == CO - 1))
                nc.scalar.activation(out=h1t[:, o, :btok], in_=ps[:, :btok],
                                     func=AF.Gelu)
            # fc2 token-major + residual
            for it in range(_ceil(btok, P)):
                x_t, rows = xts[it]
                o_t = otp.tile([P, C], F32, tag="o_t8")
                for half, fcols in ((0, 512), (1, 256)):
                    ps = m2p.tile([P, 512], F32, tag="p8ps2")
                    for ho in range(HIDO):
                        nc.tensor.matmul(
                            ps[:rows, :fcols], h1t[:, ho, it * P:it * P + rows],
                            wfc2[:, ho, half * 512:half * 512 + fcols],
                            start=(ho == 0), stop=(ho == HIDO - 1))
                    nc.vector.tensor_add(
                        out=o_t[:rows, half * 512:half * 512 + fcols],
                        in0=ps[:rows, :fcols],
                        in1=x_t[:rows, half * 512:half * 512 + fcols])
                row0 = tok0 + it * P
                if btok == 33:
                    nc.gpsimd.dma_start(out[1 + row0:1 + row0 + 32, :], o_t[:32])
                    nc.gpsimd.dma_start(out[0:1, :], o_t[32:33])
                else:
                    nc.gpsimd.dma_start(out[1 + row0:1 + row0 + rows, :],
                                        o_t[:rows])


def build_nc():
    nc = bacc.Bacc("TRN2", target_bir_lowering=False, debug=False)

    x_in = nc.dram_tensor("x", (N, C), F32, kind="ExternalInput")
    w_qk_t = nc.dram_tensor("w_qk_t", (C, 2 * C), FP8, kind="ExternalInput")
    w_v_t = nc.dram_tensor("w_v_t", (C, C), FP8, kind="ExternalInput")
    w_qk_s = nc.dram_tensor("w_qk_s", (C, 2 * C), FP8, kind="ExternalInput")
    w_v_s = nc.dram_tensor("w_v_s", (C, C), FP8, kind="ExternalInput")
    w_proj_t = nc.dram_tensor("w_proj_t", (C, C), FP8, kind="ExternalInput")
    w_tfc = nc.dram_tensor("w_tfc", (C, C), FP8, kind="ExternalInput")
    w_proj_s = nc.dram_tensor("w_proj_s", (C, C), FP8, kind="ExternalInput")
    w_fc1 = nc.dram_tensor("w_fc1", (C, HID), BF16, kind="ExternalInput")
    w_fc2 = nc.dram_tensor("w_fc2", (HID, C), BF16, kind="ExternalInput")
    mask_in = nc.dram_tensor("mask_bd", (P, 512), BF16, kind="ExternalInput")
    ident_in = nc.dram_tensor("ident", (P, P), BF16, kind="ExternalInput")
    sel12 = nc.dram_tensor("sel12", (12, C), BF16, kind="ExternalInput")
    one8 = nc.dram_tensor("one8", (8, 1), F32, kind="ExternalInput")
    out = nc.dram_tensor("out", (N, C), F32, kind="ExternalOutput")
    dbg = {}
    if KDEBUG:
        for nm, shp, dt_ in (("d_qk_t", (P, 12, NT), BF16),
                             ("d_v_t", (P, 16, 12, D + 1), BF16),
                             ("d_oT_t", (P, CO, NT), FP8),
                             ("d_xs", (NS, C), F32),
                             ("d_qk_s", (P, 12, NS), BF16),
                             ("d_oT_s", (P, CO, NS), FP8),
                             ("d_y_s", (NS, C), F32),
                             ("d_cls", (1, C), F32)):
            dbg[nm] = nc.dram_tensor(nm, shp, dt_, kind="ExternalOutput")

    # v tile specs are built inside build (need xlnT handle for stationary fns)

    with tile.TileContext(nc) as tc:
        with tc.tile_pool(name="dram", bufs=1, space="DRAM") as dram, \
             tc.tile_pool(name="const", bufs=1) as const:
            xs_d = dram.tile([NS, C], F32)
            y_s = dram.tile([NS, C], F32)

            ident = const.tile([P, P], BF16)
            nc.sync.dma_start(ident, ident_in[:])
            mask_bd = const.tile([P, 512], BF16)
            nc.sync.dma_start(mask_bd, mask_in[:])
            eps_t = const.tile([P, 1], F32)
            nc.vector.memset(eps_t, EPS)
            sel_sb = const.tile([12, C], BF16)
            nc.sync.dma_start(sel_sb, sel12[:])

            wfc1 = const.tile([P, CO, HID], BF16)
            wfc2 = const.tile([P, HIDO, C], BF16)
            nc.gpsimd.dma_start(wfc1, w_fc1[:].rearrange("(co p) o -> p co o", p=P))
            nc.gpsimd.dma_start(wfc2, w_fc2[:].rearrange("(ho p) o -> p ho o", p=P))

            ycls_sb = const.tile([8, C], F32)
            cls_row = const.tile([1, C], F32)

            oT_ctx = ExitStack()
            oT_pool = oT_ctx.enter_context(tc.tile_pool(name="oT", bufs=1))
            oT_img = oT_pool.tile([P, CO, NSP], FP8)
            img_ctx = ExitStack()
            img = img_ctx.enter_context(tc.tile_pool(name="img", bufs=1))
            qk_img = img.tile([P, 12, NS], BF16)
            v_img = img.tile([P, 16, 12, D + 1], BF16)
            xlnT = img.tile([P, CO, NSP], FP8)
            projT = img.tile([P, CO, NT], FP8)

            # ones column in every v slot (col D of each head slot)
            if KDEBUG:
                nc.vector.memset(v_img, 1.0)
            else:
                nc.vector.memset(v_img[:, :, :, D:D + 1], 1.0)

            def mk_stat(row0, rows):
                return lambda ch: xlnT[:, 2 * ch:2 * ch + 2, row0:row0 + rows]

            xlnT_r = xlnT[:].rearrange("p o (j f) -> p o j f", f=8)  # j=198 padded

            def mk_stat_f(f, c0, chlen):
                return lambda ch: xlnT_r[:, 2 * ch:2 * ch + 2, c0:c0 + chlen, f]

            v_tiles_t = [(i, min(P, NT - i * P), mk_stat(i * P, min(P, NT - i * P)))
                         for i in range(_ceil(NT, P))]
            v_tiles_s = []
            for f in range(8):
                v_tiles_s.append((2 * f, P, mk_stat_f(f, 0, P)))
                v_tiles_s.append((2 * f + 1, NSEQ - P, mk_stat_f(f, P, NSEQ - P)))

            if "1" in PHASES:
                with ExitStack() as ctx:
                    def src_t(x_t, i, rows):
                        nc.sync.dma_start(x_t[:rows],
                                          x_in[1 + i * P:1 + i * P + rows, :])
                    qkv_common(nc, tc, ctx, src_t, NT, w_qk_t, w_v_t,
                               qk_img, v_img, v_tiles_t, ident, eps_t, xlnT)
            if "2" in PHASES:
                temporal_attn(nc, tc, qk_img, v_img, mask_bd, sel_sb, oT_img)
                if KDEBUG:
                    nc.sync.dma_start(dbg["d_oT_t"][:], oT_img[:, :, :NT])
            if KDEBUG and "1" in PHASES:
                nc.sync.dma_start(dbg["d_qk_t"][:], qk_img[:, :, :NT])
                nc.sync.dma_start(dbg["d_v_t"][:], v_img)
            if "3" in PHASES:
                temporal_proj(nc, tc, x_in, w_proj_t, w_tfc, oT_img, projT, xs_d)
                if KDEBUG:
                    nc.sync.dma_start(dbg["d_xs"][:], xs_d[:])
            if "4" in PHASES:
                with ExitStack() as ctx:
                    def src_s(x_t, i, rows):
                        nc.sync.dma_start(x_t[:rows],
                                          xs_d[i * P:i * P + rows, :])
                    qkv_common(nc, tc, ctx, src_s, NS, w_qk_s, w_v_s,
                               qk_img, v_img, v_tiles_s, ident, eps_t, xlnT)
                if KDEBUG:
                    nc.sync.dma_start(dbg["d_qk_s"][:], qk_img)
            if "5" in PHASES:
                spatial_attn(nc, tc, qk_img, v_img, sel_sb, oT_img)
                if KDEBUG:
                    nc.sync.dma_start(dbg["d_oT_s"][:], oT_img[:, :, :NS])
            if "6" in PHASES:
                spatial_proj(nc, tc, xs_d, w_proj_s, oT_img, y_s, ycls_sb)
                if KDEBUG:
                    nc.sync.dma_start(dbg["d_y_s"][:], y_s[:])
            img_ctx.close()
            if "7" in PHASES:
                cls_mean(nc, tc, ycls_sb, one8, cls_row)
                if KDEBUG:
                    nc.sync.dma_start(dbg["d_cls"][:], cls_row[:])
            oT_ctx.close()
            if "8" in PHASES:
                mlp(nc, tc, y_s, cls_row, wfc1, wfc2, out, ident, eps_t)

    nc.compile()
    return nc


_NC_CACHE = None


def _get_nc():
    global _NC_CACHE
    if _NC_CACHE is None:
        _NC_CACHE = build_nc()
    return _NC_CACHE


def make_consts():
    idx = np.arange(P)
    mask = (idx[:, None] // T == idx[None, :] // T).astype(np.float32)
    mask = np.tile(mask, (1, 4))
    ident = np.eye(P, dtype=np.float32)
    sel = np.zeros((12, C), np.float32)
    for pr in range(6):
        for p in range(P):
            sel[2 * pr + p // D, pr * P + p] = 1.0 / WS
    one8 = np.full((8, 1), 0.125, np.float32)
    return mask, ident, sel, one8


def host_inputs(inputs):
    bf = lambda a: np.ascontiguousarray(np.asarray(a, np.float32).T).astype(
        ml_dtypes.bfloat16)
    f8 = lambda a: np.clip(np.ascontiguousarray(np.asarray(a, np.float32).T) * WS,
                           -240, 240).astype(ml_dtypes.float8_e4m3fn)
    f84 = lambda a: np.clip(np.ascontiguousarray(np.asarray(a, np.float32).T) * 4,
                            -240, 240).astype(ml_dtypes.float8_e4m3fn)
    qkv_w = np.asarray(inputs["qkv_w"], np.float32)
    tqkv_w = np.asarray(inputs["tqkv_w"], np.float32)
    mask, ident, sel, one8 = make_consts()
    return {
        "w_qk_t": f8(tqkv_w[:2 * C]), "w_v_t": f8(tqkv_w[2 * C:]),
        "w_qk_s": f8(qkv_w[:2 * C]), "w_v_s": f8(qkv_w[2 * C:]),
        "w_proj_t": f8(inputs["tproj_w"]),
        "w_tfc": f84(inputs["tfc_w"]),
        "w_proj_s": f8(inputs["proj_w"]),
        "w_fc1": bf(inputs["fc1_w"]), "w_fc2": bf(inputs["fc2_w"]),
        "mask_bd": mask.astype(ml_dtypes.bfloat16),
        "ident": ident.astype(ml_dtypes.bfloat16),
        "sel12": sel.astype(ml_dtypes.bfloat16),
        "one8": one8,
    }


def kernel(**inputs):
    x = np.ascontiguousarray(np.asarray(inputs["x"], dtype=np.float32))
    B = x.shape[0]
    shared = host_inputs(inputs)
    nc = _get_nc()
    in_maps = [dict(shared, x=np.ascontiguousarray(x[b])) for b in range(B)]
    res = run_bass_kernel_spmd(nc, in_maps, core_ids=list(range(B)),
                               trace=bool(int(os.environ.get("KTRACE", "0"))))
    out = np.stack([res.results[b]["out"] for b in range(B)], axis=0)
    kernel.last_results = res
    return out
